# revision 11
# baseline (speedup 1.0000x reference)
"""Distributed multi-head attention block on 8 TRN2 NeuronCores (v2).

Reference computation (B=2, S=2048, D=1024, H=16, DH=64):
    q = split_heads(q_ @ Wq + bq); k = ...; v = ...
    attn = softmax(q k^T / 8)  (mask is all-ones -> identity row mask)
    out = (merge_heads(attn @ v) + q_) @ Wf + bf

Sharding: 16 heads split 8 ways (2 heads / core); each core handles BOTH
batches.  Virtual q axis is b-major: vq = b*2048 + s (4096 total).

v2 engine plan (per core c, heads 2c/2c+1, d-slice 128c..128c+128):
  - All projections / QK / PV run fp8e4m3 DoubleRow on the PE (2x).
  - Q/K stored in a 32-partition "plane" layout so QK contracts dh=64 as
    2 DoubleRow subtiles of 32.
  - V stored with a ones column appended -> PV matmul also produces the
    softmax denominator row for free.
  - exp is split between ScalarE (true Exp activation -> fp8) and
    VectorE (int8 exponent-bit trick: round(8*log2e*s + 56) bitcast to
    fp8e4m3); softmax renormalization absorbs the approximation.
  - normalize: ScalarE drains OT psum->SBUF bf16, VectorE reciprocal,
    GpSimd broadcasts the reciprocal and multiplies -> fp8 ZT.
  - One 8-core AllToAll (fp8, 512KB) exchanges q-chunks; the residual
    half of the fc (xres^T @ Wf, bf16) runs during the A2A window; the
    attention half (zfull^T @ Wf8) is fp8 DoubleRow after it.

Host: casts/transposes inputs, feeds per-core shards, places each
core's [512, 1024] output chunk, adds bf.  Non-all-ones mask falls back
to a numpy reference (never happens with this generator).
"""

import sys

sys.path.insert(0, "/opt/trn_rl_repo")

import ml_dtypes
import numpy as np

import concourse.bass as bass
import concourse.tile as tile
from concourse import bacc, mybir
from concourse.bass_utils import run_bass_kernel_spmd

B, S, D, H = 2, 2048, 1024, 16
DH = D // H  # 64
N_CORES = 8
VQ = B * S  # 4096
NQC = VQ // 512  # 8 q-chunks
NKT = S // 128  # 16 k-tiles per batch
NPAIR = NKT // 2  # 8 k-tile pairs
NDIN = D // 128  # 8 din tiles

BF16 = mybir.dt.bfloat16
FP8 = mybir.dt.float8e4
I8 = mybir.dt.int8
F32 = mybir.dt.float32
AF = mybir.ActivationFunctionType
ALU = mybir.AluOpType
DR = mybir.MatmulPerfMode.DoubleRow
BF16NP = ml_dtypes.bfloat16
FP8NP = ml_dtypes.float8_e4m3

WSCALE = 32.0  # fp8 weight scale for q/k/v projections
W2 = 32.0  # fp8 scale for Wf in the attention-half fc
CEXP = 1.0 / (8.0 * WSCALE * WSCALE)  # score scale folded into exp
LOG2E = 1.4426950408889634
EXP_C1 = float(CEXP * 8.0 * LOG2E)  # DVE bit-trick multiplier
EXP_C2 = 56.0  # e4m3 exponent bias * 8

# kt indices handled by ScalarE's true exp; the rest use the DVE trick.
ACT_KT = [kt for kt in range(NKT) if kt % 2 == 0]

_CACHE = {}


def _build():
    nc = bacc.Bacc(None, target_bir_lowering=False)

    xq = nc.declare_dram_parameter("xq", [D, VQ], FP8, isOutput=False)
    xk = nc.declare_dram_parameter("xk", [D, VQ], FP8, isOutput=False)
    xv = nc.declare_dram_parameter("xv", [D, VQ], FP8, isOutput=False)
    wq = nc.declare_dram_parameter("wq", [D, 128], FP8, isOutput=False)
    wk = nc.declare_dram_parameter("wk", [D, 128], FP8, isOutput=False)
    wv = nc.declare_dram_parameter("wv", [D, 128], FP8, isOutput=False)
    bqk = nc.declare_dram_parameter("bqk", [64, 4], F32, isOutput=False)
    bv = nc.declare_dram_parameter("bv", [1, 128], BF16, isOutput=False)
    wf = nc.declare_dram_parameter("wf", [D, D], BF16, isOutput=False)
    wf8 = nc.declare_dram_parameter("wf8", [D, D], FP8, isOutput=False)
    xres = nc.declare_dram_parameter("xres", [D, 512], BF16, isOutput=False)
    out = nc.declare_dram_parameter("out", [512, D], F32, isOutput=True)

    with tile.TileContext(nc) as tc:
        with (
            tc.tile_pool(name="persist", bufs=1) as sbp,
            tc.tile_pool(name="dram", bufs=1, space="DRAM") as dram,
        ):
            # ---- persistent SBUF ----
            qt8 = sbp.tile([64, 2, VQ], FP8)  # [2h x 32dh, plane, vq]
            kt8 = sbp.tile([64, 2, VQ], FP8)
            v8 = sbp.tile([128, 2 * NPAIR, 2, 160], FP8)  # (b,pair), sub, 2x80/head
            wq_sb = sbp.tile([128, NDIN, 128], FP8)
            wk_sb = sbp.tile([128, NDIN, 128], FP8)
            wv_sb = sbp.tile([128, NDIN, 128], FP8)
            bqk_sb = sbp.tile([64, 4], F32)
            bv_sb = sbp.tile([1, 128], BF16)
            bvb_sb = sbp.tile([128, 128], BF16)  # bv broadcast across partitions
            ones_bf = sbp.tile([1, 128], BF16)
            nc.vector.memset(ones_bf[:], 1.0)
            # rowsum ones columns: offset 64 within each 80-wide head slot
            nc.vector.memset(
                v8[:].rearrange("p a s (h n) -> p (a s h) n", n=80)[:, :, 64:65], 1.0
            )

            a2a_in = dram.tile([1024, 512], FP8)
            a2a_out = dram.tile([1024, 512], FP8)

            # ---- input DMAs ----
            # sync queue: big x streams, kv-first so attention can start early
            xst = tc.alloc_tile_pool(name="xst", bufs=2)
            xk_b, xv_b, xq_b = [], [], []
            tiles = {}
            for b in range(B):
                for nm, src, lst in (
                    ("xk", xk, xk_b),
                    ("xv", xv, xv_b),
                    ("xq", xq, xq_b),
                ):
                    t = xst.tile([128, NDIN, 2048], FP8, name=f"{nm}{b}", tag=nm)
                    lst.append(t)
                    tiles[(nm, b)] = (t, src)
            for b in range(B):
                for nm in ("xk", "xv", "xq"):
                    t, src = tiles[(nm, b)]
                    nc.sync.dma_start(
                        t[:],
                        src[:, 2048 * b : 2048 * (b + 1)].rearrange(
                            "(t p) v -> p t v", p=128
                        ),
                    )
            # scalar queue: weights/biases
            nc.scalar.dma_start(wk_sb[:], wk[:].rearrange("(t p) n -> p t n", p=128))
            nc.scalar.dma_start(wv_sb[:], wv[:].rearrange("(t p) n -> p t n", p=128))
            nc.scalar.dma_start(wq_sb[:], wq[:].rearrange("(t p) n -> p t n", p=128))
            nc.scalar.dma_start(bqk_sb[:], bqk[:])
            nc.scalar.dma_start(bv_sb[:], bv[:])
            nc.gpsimd.partition_broadcast(bvb_sb[:], bv_sb[:])

            # late tensors (fc stage); DMAs queue behind the x streams
            wf_sb = sbp.tile([128, NDIN, 1024], BF16)
            wf8_sb = sbp.tile([128, NDIN, 1024], FP8)
            xres_sb = sbp.tile([128, NDIN, 512], BF16)
            yres_sb = sbp.tile([128, 4, 1024], BF16)
            zf_sb = sbp.tile([128, NDIN, 512], FP8)
            nc.scalar.dma_start(xres_sb[:], xres[:].rearrange("(t p) n -> p t n", p=128))
            nc.scalar.dma_start(wf_sb[:], wf[:].rearrange("(t p) n -> p t n", p=128))
            nc.scalar.dma_start(wf8_sb[:], wf8[:].rearrange("(t p) n -> p t n", p=128))

            # ---- projection helpers ----
            def w_pair(w_sb, dp, pl):
                # lhsT [128, 2, 64]: din tiles (2dp, 2dp+1), plane cols
                return w_sb[:, 2 * dp : 2 * dp + 2, 64 * pl : 64 * pl + 64]

            def w_pair_full(w_sb, dp):
                # lhsT [128, 2, 128]: full 128 dout cols (V projection)
                return w_sb[:, 2 * dp : 2 * dp + 2, :]

            def qk_proj_emit(qkps, dst, w_sb, bcol, xt, b, win, copy_engine):
                """One 512-q window of the Q or K projection (DoubleRow)."""
                q0 = 2048 * b + 512 * win
                for pl in range(2):
                    ps = qkps.tile([64, 512], F32, name=f"qkp{b}_{win}_{pl}", tag="qkps")
                    for dp in range(4):
                        nc.tensor.matmul(
                            ps[:],
                            lhsT=w_pair(w_sb, dp, pl),
                            rhs=xt[:, 2 * dp : 2 * dp + 2, 512 * win : 512 * win + 512],
                            start=(dp == 0),
                            stop=(dp == 3),
                            perf_mode=DR,
                        )
                    if copy_engine == "act":
                        nc.scalar.activation(
                            dst[0:64, pl, q0 : q0 + 512],
                            ps[:],
                            AF.Identity,
                            bias=bqk_sb[:, bcol + pl : bcol + pl + 1],
                        )
                    else:
                        nc.vector.tensor_scalar_add(
                            dst[0:64, pl, q0 : q0 + 512],
                            ps[:],
                            bqk_sb[:, bcol + pl : bcol + pl + 1],
                        )

            def v_unit(vps, b, kt):
                vp = vps.tile([128, 128], F32, name=f"vp{b}_{kt}", tag="vps")
                for dp in range(4):
                    nc.tensor.matmul(
                        vp[:],
                        lhsT=w_pair_full(wv_sb, dp),
                        rhs=xv_b[b][:, 2 * dp : 2 * dp + 2, 128 * kt : 128 * (kt + 1)],
                        start=(dp == 0),
                        stop=(dp == 3),
                        perf_mode=DR,
                    )
                slot = NPAIR * b + kt // 2
                nc.vector.tensor_tensor(
                    v8[:, slot, kt % 2, :].rearrange("p (h n) -> p h n", h=2)[
                        :, :, 0:64
                    ],
                    vp[:].rearrange("p (h n) -> p h n", n=64),
                    bvb_sb[:].rearrange("p (h n) -> p h n", n=64),
                    ALU.add,
                )

            # ================= phase 1: batch-0 projections =================
            with (
                tc.tile_pool(name="qkps1", bufs=2, space="PSUM") as qkps1,
                tc.tile_pool(name="vps1", bufs=2, space="PSUM") as vps1,
            ):
                warm = vps1.tile([64, 64], F32, name="warm", tag="warm")
                for _ in range(100):
                    nc.tensor.matmul(
                        warm[:], lhsT=ones_bf[0:1, 0:64], rhs=ones_bf[0:1, 0:64],
                        start=True, stop=True,
                    )
                for win in range(4):
                    qk_proj_emit(qkps1, kt8, wk_sb, 2, xk_b[0], 0, win, "act")
                for kt in range(NKT):
                    v_unit(vps1, 0, kt)
                for win in range(4):
                    qk_proj_emit(qkps1, qt8, wq_sb, 0, xq_b[0], 0, win, "act")

            # ================= phase 2: attention =================
            with (
                tc.tile_pool(name="stp", bufs=2, space="PSUM") as stp,
                tc.tile_pool(name="ptp", bufs=10) as ptp,
                tc.tile_pool(name="nrm", bufs=2) as nrm,
                tc.tile_pool(name="ztp", bufs=4) as ztp,
            ):
                def emit_qk(qc, kt, pt):
                    """Scores for (qc, kt) -> exp into pt[:, kt%2, :]."""
                    b = qc // 4
                    q0 = 512 * qc
                    kk = 2048 * b + 128 * kt
                    st = stp.tile([128, 1024], F32, name=f"st{qc}_{kt}", tag="st")
                    for h in range(2):
                        nc.tensor.matmul(
                            st[:, 512 * h : 512 * (h + 1)],
                            lhsT=kt8[32 * h : 32 * h + 32, :, kk : kk + 128],
                            rhs=qt8[32 * h : 32 * h + 32, :, q0 : q0 + 512],
                            start=True,
                            stop=True,
                            perf_mode=DR,
                        )
                    if kt in ACT_KT:
                        nc.scalar.activation(
                            pt[:, kt % 2, :], st[:], AF.Exp, scale=CEXP
                        )
                    else:
                        nc.vector.tensor_scalar(
                            pt[:, kt % 2, :].bitcast(I8),
                            st[:],
                            EXP_C1,
                            EXP_C2,
                            ALU.mult,
                            ALU.add,
                        )

                def emit_qk_pair(qc, pair):
                    pt = ptp.tile([128, 2, 1024], FP8, name=f"pt{qc}_{pair}", tag="pt")
                    emit_qk(qc, 2 * pair, pt)
                    emit_qk(qc, 2 * pair + 1, pt)
                    return pt

                def emit_pv(qc, pair, pt, ot0, ot1):
                    b = qc // 4
                    slot = NPAIR * b + pair
                    for h, ot in ((0, ot0), (1, ot1)):
                        nc.tensor.matmul(
                            ot[:],
                            lhsT=v8[:, slot, :, 80 * h : 80 * h + 65],
                            rhs=pt[:, :, 512 * h : 512 * (h + 1)],
                            start=(pair == 0),
                            stop=(pair == NPAIR - 1),
                            perf_mode=DR,
                        )

                def normalize(qc, ot0, ot1):
                    q0 = 512 * qc
                    osb = nrm.tile([65, 1024], BF16, name=f"osb{qc}", tag="osb")
                    nc.scalar.copy(osb[:, 0:512], ot0[:])
                    nc.scalar.copy(osb[:, 512:1024], ot1[:])
                    rb = nrm.tile([1, 1024], BF16, name=f"rb{qc}", tag="rb")
                    with nc.allow_low_precision(reason="softmax denom recip"):
                        nc.vector.reciprocal(rb[:], osb[64:65, :])
                    zbc = nrm.tile([64, 1024], BF16, name=f"zbc{qc}", tag="zbc")
                    nc.gpsimd.partition_broadcast(zbc[:], rb[:])
                    for h in range(2):
                        zt = ztp.tile([64, 512], FP8, name=f"zt{qc}_{h}", tag="zt")
                        nc.gpsimd.tensor_tensor(
                            zt[:],
                            osb[0:64, 512 * h : 512 * (h + 1)],
                            zbc[:, 512 * h : 512 * (h + 1)],
                            ALU.mult,
                        )
                        nc.gpsimd.dma_start(
                            a2a_in[128 * qc + 64 * h : 128 * qc + 64 * h + 64, :],
                            zt[:],
                        )

                # batch-1 projection units, spread across qc0..qc2 windows
                b1_units = []
                with (
                    tc.tile_pool(name="qkps2", bufs=1, space="PSUM") as qkps2,
                    tc.tile_pool(name="vps2", bufs=1, space="PSUM") as vps2,
                ):
                    for win in range(4):
                        b1_units.append(
                            lambda w=win: qk_proj_emit(
                                qkps2, kt8, wk_sb, 2, xk_b[1], 1, w, "act"
                            )
                        )
                    for kt in range(0, NKT, 2):
                        b1_units.append(lambda k=kt: (v_unit(vps2, 1, k), v_unit(vps2, 1, k + 1)))
                    for win in range(4):
                        b1_units.append(
                            lambda w=win: qk_proj_emit(
                                qkps2, qt8, wq_sb, 0, xq_b[1], 1, w, "dve"
                            )
                        )
                    ui = 0

                    # prologue: qc0 scores interleaved with early b1 units
                    pts = []
                    for pair in range(NPAIR):
                        pts.append(emit_qk_pair(0, pair))
                        if pair % 2 == 1 and ui < 4:
                            b1_units[ui]()
                            ui += 1

                    otp = tc.alloc_tile_pool(name="otp", bufs=1, space="PSUM")
                    pending = None
                    for qc in range(NQC):
                        ot0 = otp.tile([65, 512], F32, name=f"ot0_{qc}", tag="ot0")
                        ot1 = otp.tile([65, 512], F32, name=f"ot1_{qc}", tag="ot1")
                        nxt = []
                        for pair in range(NPAIR):
                            emit_pv(qc, pair, pts[pair], ot0, ot1)
                            if qc + 1 < NQC:
                                nxt.append(emit_qk_pair(qc + 1, pair))
                            if pair == 1 and pending is not None:
                                normalize(*pending)
                                pending = None
                            if pair % 2 == 1 and ui < len(b1_units):
                                b1_units[ui]()
                                ui += 1
                        if pending is not None:
                            normalize(*pending)
                        pending = (qc, ot0, ot1)
                        pts = nxt
                    normalize(*pending)
                    otp.release()

            xst.release()

            # ================= phase 3: A2A + fc =================
            nc.gpsimd.collective_compute(
                "AllToAll",
                ALU.bypass,
                replica_groups=[list(range(N_CORES))],
                ins=[a2a_in.opt()],
                outs=[a2a_out.opt()],
            )

            # residual half of fc (bf16) rides the A2A window
            with (
                tc.tile_pool(name="fcps", bufs=4, space="PSUM") as fcps,
                tc.tile_pool(name="ysb", bufs=2) as ysb,
            ):
                for qt in range(4):
                    for nb in range(2):
                        yp = fcps.tile([128, 512], F32, name=f"yr{qt}_{nb}", tag="yr")
                        for j in range(NDIN):
                            nc.tensor.matmul(
                                yp[:],
                                lhsT=xres_sb[:, j, 128 * qt : 128 * (qt + 1)],
                                rhs=wf_sb[:, j, 512 * nb : 512 * (nb + 1)],
                                start=(j == 0),
                                stop=(j == NDIN - 1),
                            )
                        nc.vector.tensor_copy(
                            yres_sb[:, qt, 512 * nb : 512 * (nb + 1)], yp[:]
                        )

                nc.sync.dma_start(
                    zf_sb[:], a2a_out[:].rearrange("(t p) v -> p t v", p=128)
                )
                # attention half (fp8 DoubleRow) + residual add + store
                for qt in range(4):
                    y = ysb.tile([128, 1024], F32, name=f"y{qt}", tag="y")
                    for nb in range(2):
                        yp = fcps.tile([128, 512], F32, name=f"ya{qt}_{nb}", tag="ya")
                        for dp in range(4):
                            nc.tensor.matmul(
                                yp[:],
                                lhsT=zf_sb[:, 2 * dp : 2 * dp + 2, 128 * qt : 128 * (qt + 1)],
                                rhs=wf8_sb[:, 2 * dp : 2 * dp + 2, 512 * nb : 512 * (nb + 1)],
                                start=(dp == 0),
                                stop=(dp == 3),
                                perf_mode=DR,
                            )
                        nc.vector.scalar_tensor_tensor(
                            y[:, 512 * nb : 512 * (nb + 1)],
                            yp[:],
                            1.0 / (WSCALE * W2),
                            yres_sb[:, qt, 512 * nb : 512 * (nb + 1)],
                            ALU.mult,
                            ALU.add,
                        )
                    nc.sync.dma_start(out[128 * qt : 128 * (qt + 1), :], y[:])

    nc.compile()
    return nc


def _numpy_reference(q_, k_, v_, mask, Wq, bq, Wk, bk, Wv, bv, Wf, bf):
    q_ = np.asarray(q_, np.float32)
    k_ = np.asarray(k_, np.float32)
    v_ = np.asarray(v_, np.float32)
    b = q_.shape[0]

    def split(x):
        return x.reshape(b, -1, H, DH).transpose(0, 2, 1, 3)

    q = split(q_ @ Wq + bq)
    k = split(k_ @ Wk + bk)
    v = split(v_ @ Wv + bv)
    attn = np.einsum("bhqd,bhkd->bhqk", q, k) / np.sqrt(np.float32(DH))
    attn = np.where(np.asarray(mask)[:, None, :, None], attn, np.float32(-1e12))
    attn = attn - attn.max(axis=-1, keepdims=True)
    e = np.exp(attn)
    p = e / e.sum(axis=-1, keepdims=True)
    o = np.einsum("bhqk,bhkd->bhqd", p, v)
    o = o.transpose(0, 2, 1, 3).reshape(b, -1, D)
    return (o + q_) @ Wf + bf


# plane-permuted column order for Wq/Wk: [h0 dh0-31, h1 dh0-31, h0 dh32-63, ...]
_PERM = np.concatenate(
    [np.arange(0, 32), np.arange(64, 96), np.arange(32, 64), np.arange(96, 128)]
)


def kernel(q_, k_, v_, mask, Wq, bq, Wk, bk, Wv, bv, Wf, bf):
    mask = np.asarray(mask)
    if not mask.all():
        return _numpy_reference(q_, k_, v_, mask, Wq, bq, Wk, bk, Wv, bv, Wf, bf)

    q_ = np.asarray(q_, np.float32)
    k_ = np.asarray(k_, np.float32)
    v_ = np.asarray(v_, np.float32)
    Wq = np.asarray(Wq, np.float32)
    Wk = np.asarray(Wk, np.float32)
    Wv = np.asarray(Wv, np.float32)
    Wf = np.asarray(Wf, np.float32)
    bq = np.asarray(bq, np.float32)
    bk = np.asarray(bk, np.float32)
    bv = np.asarray(bv, np.float32)

    xq_c = np.ascontiguousarray(np.concatenate([q_[b].T for b in range(B)], axis=1))
    xq8 = xq_c.astype(FP8NP)
    xk8 = np.ascontiguousarray(
        np.concatenate([k_[b].T for b in range(B)], axis=1)
    ).astype(FP8NP)
    xv8 = np.ascontiguousarray(
        np.concatenate([v_[b].T for b in range(B)], axis=1)
    ).astype(FP8NP)
    wf_b = np.ascontiguousarray(Wf).astype(BF16NP)
    wf8_b = np.ascontiguousarray(Wf * W2).astype(FP8NP)

    in_maps = []
    for c in range(N_CORES):
        d0 = 128 * c
        perm = d0 + _PERM
        # per-plane biases: partition p of plane pl holds dout perm[64*pl + p]
        bqk = np.stack(
            [
                bq[perm[0:64]] * WSCALE,
                bq[perm[64:128]] * WSCALE,
                bk[perm[0:64]] * WSCALE,
                bk[perm[64:128]] * WSCALE,
            ],
            axis=1,
        )
        in_maps.append(
            {
                "xq": xq8,
                "xk": xk8,
                "xv": xv8,
                "wq": np.ascontiguousarray(Wq[:, perm] * WSCALE).astype(FP8NP),
                "wk": np.ascontiguousarray(Wk[:, perm] * WSCALE).astype(FP8NP),
                "wv": np.ascontiguousarray(
                    Wv[:, d0 : d0 + 128] * WSCALE
                ).astype(FP8NP),
                "bqk": np.ascontiguousarray(bqk),
                "bv": np.ascontiguousarray(
                    bv[None, d0 : d0 + 128] * WSCALE
                ).astype(BF16NP),
                "wf": wf_b,
                "wf8": wf8_b,
                "xres": np.ascontiguousarray(
                    q_[c // 4].T[:, 512 * (c % 4) : 512 * (c % 4 + 1)]
                ).astype(BF16NP),
            }
        )

    if "nc" not in _CACHE:
        _CACHE["nc"] = _build()
    res = run_bass_kernel_spmd(_CACHE["nc"], in_maps, core_ids=list(range(N_CORES)))

    out = np.empty((B, S, D), np.float32)
    for c in range(N_CORES):
        y = res.results[c]["out"]
        out[c // 4, 512 * (c % 4) : 512 * (c % 4 + 1), :] = y
    out += np.asarray(bf, np.float32)[None, None, :]
    return out


if __name__ == "__main__":
    rng = np.random.default_rng(0)
    args = dict(
        q_=rng.standard_normal((B, S, D), dtype=np.float32),
        k_=rng.standard_normal((B, S, D), dtype=np.float32),
        v_=rng.standard_normal((B, S, D), dtype=np.float32),
        mask=np.ones((B, S), bool),
        Wq=rng.standard_normal((D, D), dtype=np.float32) * 0.02,
        bq=np.zeros(D, np.float32),
        Wk=rng.standard_normal((D, D), dtype=np.float32) * 0.02,
        bk=np.zeros(D, np.float32),
        Wv=rng.standard_normal((D, D), dtype=np.float32) * 0.02,
        bv=np.zeros(D, np.float32),
        Wf=rng.standard_normal((D, D), dtype=np.float32) * 0.02,
        bf=np.zeros(D, np.float32),
    )
    got = kernel(**args)
    want = _numpy_reference(**args)
    rel = np.abs(got - want).max() / np.abs(want).max()
    print("rel_err:", rel)


# revision 12
# speedup vs baseline: 1.0038x; 1.0038x over previous
"""Distributed multi-head attention block on 8 TRN2 NeuronCores (v2).

Reference computation (B=2, S=2048, D=1024, H=16, DH=64):
    q = split_heads(q_ @ Wq + bq); k = ...; v = ...
    attn = softmax(q k^T / 8)  (mask is all-ones -> identity row mask)
    out = (merge_heads(attn @ v) + q_) @ Wf + bf

Sharding: 16 heads split 8 ways (2 heads / core); each core handles BOTH
batches.  Virtual q axis is b-major: vq = b*2048 + s (4096 total).

v2 engine plan (per core c, heads 2c/2c+1, d-slice 128c..128c+128):
  - All projections / QK / PV run fp8e4m3 DoubleRow on the PE (2x).
  - Q/K stored in a 32-partition "plane" layout so QK contracts dh=64 as
    2 DoubleRow subtiles of 32.
  - V stored with a ones column appended -> PV matmul also produces the
    softmax denominator row for free.
  - exp is split between ScalarE (true Exp activation -> fp8) and
    VectorE (int8 exponent-bit trick: round(8*log2e*s + 56) bitcast to
    fp8e4m3); softmax renormalization absorbs the approximation.
  - normalize: ScalarE drains OT psum->SBUF bf16, VectorE reciprocal,
    GpSimd broadcasts the reciprocal and multiplies -> fp8 ZT.
  - One 8-core AllToAll (fp8, 512KB) exchanges q-chunks; the residual
    half of the fc (xres^T @ Wf, bf16) runs during the A2A window; the
    attention half (zfull^T @ Wf8) is fp8 DoubleRow after it.

Host: casts/transposes inputs, feeds per-core shards, places each
core's [512, 1024] output chunk, adds bf.  Non-all-ones mask falls back
to a numpy reference (never happens with this generator).
"""

import sys

sys.path.insert(0, "/opt/trn_rl_repo")

import ml_dtypes
import numpy as np

import concourse.bass as bass
import concourse.tile as tile
from concourse import bacc, mybir
from concourse.bass_utils import run_bass_kernel_spmd

B, S, D, H = 2, 2048, 1024, 16
DH = D // H  # 64
N_CORES = 8
VQ = B * S  # 4096
NQC = VQ // 512  # 8 q-chunks
NKT = S // 128  # 16 k-tiles per batch
NPAIR = NKT // 2  # 8 k-tile pairs
NDIN = D // 128  # 8 din tiles

BF16 = mybir.dt.bfloat16
FP8 = mybir.dt.float8e4
I8 = mybir.dt.int8
F32 = mybir.dt.float32
AF = mybir.ActivationFunctionType
ALU = mybir.AluOpType
DR = mybir.MatmulPerfMode.DoubleRow
BF16NP = ml_dtypes.bfloat16
FP8NP = ml_dtypes.float8_e4m3

WSCALE = 32.0  # fp8 weight scale for q/k/v projections
W2 = 32.0  # fp8 scale for Wf in the attention-half fc
CEXP = 1.0 / (8.0 * WSCALE * WSCALE)  # score scale folded into exp
LOG2E = 1.4426950408889634
EXP_C1 = float(CEXP * 8.0 * LOG2E)  # DVE bit-trick multiplier
EXP_C2 = 56.0  # e4m3 exponent bias * 8

# kt indices handled by ScalarE's true exp; the rest use the DVE trick.
ACT_KT = [kt for kt in range(NKT) if kt % 2 == 0]

_CACHE = {}


def _build():
    nc = bacc.Bacc(None, target_bir_lowering=False)

    xq = nc.declare_dram_parameter("xq", [D, VQ], FP8, isOutput=False)
    xk = nc.declare_dram_parameter("xk", [D, VQ], FP8, isOutput=False)
    xv = nc.declare_dram_parameter("xv", [D, VQ], FP8, isOutput=False)
    wq = nc.declare_dram_parameter("wq", [D, 128], FP8, isOutput=False)
    wk = nc.declare_dram_parameter("wk", [D, 128], FP8, isOutput=False)
    wv = nc.declare_dram_parameter("wv", [D, 128], FP8, isOutput=False)
    bqk = nc.declare_dram_parameter("bqk", [64, 4], F32, isOutput=False)
    bv = nc.declare_dram_parameter("bv", [1, 128], BF16, isOutput=False)
    wf = nc.declare_dram_parameter("wf", [D, D], BF16, isOutput=False)
    wf8 = nc.declare_dram_parameter("wf8", [D, D], FP8, isOutput=False)
    xres = nc.declare_dram_parameter("xres", [D, 512], BF16, isOutput=False)
    out = nc.declare_dram_parameter("out", [512, D], F32, isOutput=True)

    with tile.TileContext(nc) as tc:
        with (
            tc.tile_pool(name="persist", bufs=1) as sbp,
            tc.tile_pool(name="dram", bufs=1, space="DRAM") as dram,
        ):
            # ---- persistent SBUF ----
            qt8 = sbp.tile([64, 2, VQ], FP8)  # [2h x 32dh, plane, vq]
            kt8 = sbp.tile([64, 2, VQ], FP8)
            v8 = sbp.tile([128, 2 * NPAIR, 2, 160], FP8)  # (b,pair), sub, 2x80/head
            wq_sb = sbp.tile([128, NDIN, 128], FP8)
            wk_sb = sbp.tile([128, NDIN, 128], FP8)
            wv_sb = sbp.tile([128, NDIN, 128], FP8)
            bqk_sb = sbp.tile([64, 4], F32)
            bv_sb = sbp.tile([1, 128], BF16)
            bvb_sb = sbp.tile([128, 128], BF16)  # bv broadcast across partitions
            ones_bf = sbp.tile([1, 128], BF16)
            nc.vector.memset(ones_bf[:], 1.0)
            # rowsum ones columns: offset 64 within each 80-wide head slot
            nc.vector.memset(
                v8[:].rearrange("p a s (h n) -> p (a s h) n", n=80)[:, :, 64:65], 1.0
            )

            a2a_in = dram.tile([1024, 512], FP8)
            a2a_out = dram.tile([1024, 512], FP8)

            # ---- input DMAs ----
            # sync queue: big x streams, kv-first so attention can start early
            xst = tc.alloc_tile_pool(name="xst", bufs=2)
            xk_b, xv_b, xq_b = [], [], []
            tiles = {}
            for b in range(B):
                for nm, src, lst in (
                    ("xk", xk, xk_b),
                    ("xv", xv, xv_b),
                    ("xq", xq, xq_b),
                ):
                    t = xst.tile([128, NDIN, 2048], FP8, name=f"{nm}{b}", tag=nm)
                    lst.append(t)
                    tiles[(nm, b)] = (t, src)
            for b in range(B):
                for nm in ("xk", "xv", "xq"):
                    t, src = tiles[(nm, b)]
                    nc.sync.dma_start(
                        t[:],
                        src[:, 2048 * b : 2048 * (b + 1)].rearrange(
                            "(t p) v -> p t v", p=128
                        ),
                    )
            # scalar queue: weights/biases
            nc.scalar.dma_start(wk_sb[:], wk[:].rearrange("(t p) n -> p t n", p=128))
            nc.scalar.dma_start(wv_sb[:], wv[:].rearrange("(t p) n -> p t n", p=128))
            nc.scalar.dma_start(wq_sb[:], wq[:].rearrange("(t p) n -> p t n", p=128))
            nc.scalar.dma_start(bqk_sb[:], bqk[:])
            nc.scalar.dma_start(bv_sb[:], bv[:])
            nc.gpsimd.partition_broadcast(bvb_sb[:], bv_sb[:])

            # late tensors (fc stage); DMAs queue behind the x streams
            wf_sb = sbp.tile([128, NDIN, 1024], BF16)
            wf8_sb = sbp.tile([128, NDIN, 1024], FP8)
            xres_sb = sbp.tile([128, NDIN, 512], BF16)
            yres_sb = sbp.tile([128, 4, 1024], BF16)
            zf_sb = sbp.tile([128, NDIN, 512], FP8)
            nc.scalar.dma_start(xres_sb[:], xres[:].rearrange("(t p) n -> p t n", p=128))
            nc.scalar.dma_start(wf_sb[:], wf[:].rearrange("(t p) n -> p t n", p=128))
            nc.scalar.dma_start(wf8_sb[:], wf8[:].rearrange("(t p) n -> p t n", p=128))

            # ---- projection helpers ----
            def w_pair(w_sb, dp, pl):
                # lhsT [128, 2, 64]: din tiles (2dp, 2dp+1), plane cols
                return w_sb[:, 2 * dp : 2 * dp + 2, 64 * pl : 64 * pl + 64]

            def w_pair_full(w_sb, dp):
                # lhsT [128, 2, 128]: full 128 dout cols (V projection)
                return w_sb[:, 2 * dp : 2 * dp + 2, :]

            def qk_proj_emit(qkps, dst, w_sb, bcol, xt, b, win, copy_engine):
                """One 512-q window of the Q or K projection (DoubleRow)."""
                q0 = 2048 * b + 512 * win
                for pl in range(2):
                    ps = qkps.tile([64, 512], F32, name=f"qkp{b}_{win}_{pl}", tag="qkps")
                    for dp in range(4):
                        nc.tensor.matmul(
                            ps[:],
                            lhsT=w_pair(w_sb, dp, pl),
                            rhs=xt[:, 2 * dp : 2 * dp + 2, 512 * win : 512 * win + 512],
                            start=(dp == 0),
                            stop=(dp == 3),
                            perf_mode=DR,
                        )
                    if copy_engine == "act":
                        nc.scalar.activation(
                            dst[0:64, pl, q0 : q0 + 512],
                            ps[:],
                            AF.Identity,
                            bias=bqk_sb[:, bcol + pl : bcol + pl + 1],
                        )
                    else:
                        nc.vector.tensor_scalar_add(
                            dst[0:64, pl, q0 : q0 + 512],
                            ps[:],
                            bqk_sb[:, bcol + pl : bcol + pl + 1],
                        )

            def v_unit(vps, b, kt):
                vp = vps.tile([128, 128], F32, name=f"vp{b}_{kt}", tag="vps")
                for dp in range(4):
                    nc.tensor.matmul(
                        vp[:],
                        lhsT=xv_b[b][:, 2 * dp : 2 * dp + 2, 128 * kt : 128 * (kt + 1)],
                        rhs=w_pair_full(wv_sb, dp),
                        start=(dp == 0),
                        stop=(dp == 3),
                        perf_mode=DR,
                    )
                slot = NPAIR * b + kt // 2
                nc.vector.tensor_tensor(
                    v8[:, slot, kt % 2, :].rearrange("p (h n) -> p h n", h=2)[
                        :, :, 0:64
                    ],
                    vp[:].rearrange("p (h n) -> p h n", n=64),
                    bvb_sb[:].rearrange("p (h n) -> p h n", n=64),
                    ALU.add,
                )

            # ================= phase 1: batch-0 projections =================
            with (
                tc.tile_pool(name="qkps1", bufs=2, space="PSUM") as qkps1,
                tc.tile_pool(name="vps1", bufs=2, space="PSUM") as vps1,
            ):
                warm = vps1.tile([64, 64], F32, name="warm", tag="warm")
                for _ in range(100):
                    nc.tensor.matmul(
                        warm[:], lhsT=ones_bf[0:1, 0:64], rhs=ones_bf[0:1, 0:64],
                        start=True, stop=True,
                    )
                for win in range(4):
                    qk_proj_emit(qkps1, kt8, wk_sb, 2, xk_b[0], 0, win, "act")
                for kt in range(NKT):
                    v_unit(vps1, 0, kt)
                for win in range(4):
                    qk_proj_emit(qkps1, qt8, wq_sb, 0, xq_b[0], 0, win, "act")

            # ================= phase 2: attention =================
            with (
                tc.tile_pool(name="stp", bufs=2, space="PSUM") as stp,
                tc.tile_pool(name="ptp", bufs=10) as ptp,
                tc.tile_pool(name="nrm", bufs=2) as nrm,
                tc.tile_pool(name="ztp", bufs=4) as ztp,
            ):
                def emit_qk(qc, kt, pt):
                    """Scores for (qc, kt) -> exp into pt[:, kt%2, :]."""
                    b = qc // 4
                    q0 = 512 * qc
                    kk = 2048 * b + 128 * kt
                    st = stp.tile([128, 1024], F32, name=f"st{qc}_{kt}", tag="st")
                    for h in range(2):
                        nc.tensor.matmul(
                            st[:, 512 * h : 512 * (h + 1)],
                            lhsT=kt8[32 * h : 32 * h + 32, :, kk : kk + 128],
                            rhs=qt8[32 * h : 32 * h + 32, :, q0 : q0 + 512],
                            start=True,
                            stop=True,
                            perf_mode=DR,
                        )
                    if kt in ACT_KT:
                        nc.scalar.activation(
                            pt[:, kt % 2, :], st[:], AF.Exp, scale=CEXP
                        )
                    else:
                        nc.vector.tensor_scalar(
                            pt[:, kt % 2, :].bitcast(I8),
                            st[:],
                            EXP_C1,
                            EXP_C2,
                            ALU.mult,
                            ALU.add,
                        )

                def emit_qk_pair(qc, pair):
                    pt = ptp.tile([128, 2, 1024], FP8, name=f"pt{qc}_{pair}", tag="pt")
                    emit_qk(qc, 2 * pair, pt)
                    emit_qk(qc, 2 * pair + 1, pt)
                    return pt

                def emit_pv(qc, pair, pt, ot0, ot1):
                    b = qc // 4
                    slot = NPAIR * b + pair
                    for h, ot in ((0, ot0), (1, ot1)):
                        nc.tensor.matmul(
                            ot[:],
                            lhsT=v8[:, slot, :, 80 * h : 80 * h + 65],
                            rhs=pt[:, :, 512 * h : 512 * (h + 1)],
                            start=(pair == 0),
                            stop=(pair == NPAIR - 1),
                            perf_mode=DR,
                        )

                def normalize(qc, ot0, ot1):
                    q0 = 512 * qc
                    osb = nrm.tile([65, 1024], BF16, name=f"osb{qc}", tag="osb")
                    nc.scalar.copy(osb[:, 0:512], ot0[:])
                    nc.scalar.copy(osb[:, 512:1024], ot1[:])
                    rb = nrm.tile([1, 1024], BF16, name=f"rb{qc}", tag="rb")
                    with nc.allow_low_precision(reason="softmax denom recip"):
                        nc.vector.reciprocal(rb[:], osb[64:65, :])
                    zbc = nrm.tile([64, 1024], BF16, name=f"zbc{qc}", tag="zbc")
                    nc.gpsimd.partition_broadcast(zbc[:], rb[:])
                    for h in range(2):
                        zt = ztp.tile([64, 512], FP8, name=f"zt{qc}_{h}", tag="zt")
                        nc.gpsimd.tensor_tensor(
                            zt[:],
                            osb[0:64, 512 * h : 512 * (h + 1)],
                            zbc[:, 512 * h : 512 * (h + 1)],
                            ALU.mult,
                        )
                        nc.gpsimd.dma_start(
                            a2a_in[128 * qc + 64 * h : 128 * qc + 64 * h + 64, :],
                            zt[:],
                        )

                # batch-1 projection units, spread across qc0..qc2 windows
                b1_units = []
                with (
                    tc.tile_pool(name="qkps2", bufs=1, space="PSUM") as qkps2,
                    tc.tile_pool(name="vps2", bufs=1, space="PSUM") as vps2,
                ):
                    for win in range(4):
                        b1_units.append(
                            lambda w=win: qk_proj_emit(
                                qkps2, kt8, wk_sb, 2, xk_b[1], 1, w, "act"
                            )
                        )
                    for kt in range(0, NKT, 2):
                        b1_units.append(lambda k=kt: (v_unit(vps2, 1, k), v_unit(vps2, 1, k + 1)))
                    for win in range(4):
                        b1_units.append(
                            lambda w=win: qk_proj_emit(
                                qkps2, qt8, wq_sb, 0, xq_b[1], 1, w, "dve"
                            )
                        )
                    ui = 0

                    # prologue: qc0 scores interleaved with early b1 units
                    pts = []
                    for pair in range(NPAIR):
                        pts.append(emit_qk_pair(0, pair))
                        if pair % 2 == 1 and ui < 4:
                            b1_units[ui]()
                            ui += 1

                    otp = tc.alloc_tile_pool(name="otp", bufs=1, space="PSUM")
                    pending = None
                    for qc in range(NQC):
                        ot0 = otp.tile([65, 512], F32, name=f"ot0_{qc}", tag="ot0")
                        ot1 = otp.tile([65, 512], F32, name=f"ot1_{qc}", tag="ot1")
                        nxt = []
                        for pair in range(NPAIR):
                            emit_pv(qc, pair, pts[pair], ot0, ot1)
                            if qc + 1 < NQC:
                                nxt.append(emit_qk_pair(qc + 1, pair))
                            if pair == 1 and pending is not None:
                                normalize(*pending)
                                pending = None
                            if pair % 2 == 1 and ui < len(b1_units):
                                b1_units[ui]()
                                ui += 1
                        if pending is not None:
                            normalize(*pending)
                        pending = (qc, ot0, ot1)
                        pts = nxt
                    normalize(*pending)
                    otp.release()

            xst.release()

            # ================= phase 3: A2A + fc =================
            nc.gpsimd.collective_compute(
                "AllToAll",
                ALU.bypass,
                replica_groups=[list(range(N_CORES))],
                ins=[a2a_in.opt()],
                outs=[a2a_out.opt()],
            )

            # residual half of fc (bf16) rides the A2A window
            with (
                tc.tile_pool(name="fcps", bufs=4, space="PSUM") as fcps,
                tc.tile_pool(name="ysb", bufs=2) as ysb,
            ):
                for qt in range(4):
                    for nb in range(2):
                        yp = fcps.tile([128, 512], F32, name=f"yr{qt}_{nb}", tag="yr")
                        for j in range(NDIN):
                            nc.tensor.matmul(
                                yp[:],
                                lhsT=xres_sb[:, j, 128 * qt : 128 * (qt + 1)],
                                rhs=wf_sb[:, j, 512 * nb : 512 * (nb + 1)],
                                start=(j == 0),
                                stop=(j == NDIN - 1),
                            )
                        nc.vector.tensor_copy(
                            yres_sb[:, qt, 512 * nb : 512 * (nb + 1)], yp[:]
                        )

                nc.sync.dma_start(
                    zf_sb[:], a2a_out[:].rearrange("(t p) v -> p t v", p=128)
                )
                # attention half (fp8 DoubleRow) + residual add + store
                for qt in range(4):
                    y = ysb.tile([128, 1024], F32, name=f"y{qt}", tag="y")
                    for nb in range(2):
                        yp = fcps.tile([128, 512], F32, name=f"ya{qt}_{nb}", tag="ya")
                        for dp in range(4):
                            nc.tensor.matmul(
                                yp[:],
                                lhsT=zf_sb[:, 2 * dp : 2 * dp + 2, 128 * qt : 128 * (qt + 1)],
                                rhs=wf8_sb[:, 2 * dp : 2 * dp + 2, 512 * nb : 512 * (nb + 1)],
                                start=(dp == 0),
                                stop=(dp == 3),
                                perf_mode=DR,
                            )
                        nc.vector.scalar_tensor_tensor(
                            y[:, 512 * nb : 512 * (nb + 1)],
                            yp[:],
                            1.0 / (WSCALE * W2),
                            yres_sb[:, qt, 512 * nb : 512 * (nb + 1)],
                            ALU.mult,
                            ALU.add,
                        )
                    nc.sync.dma_start(out[128 * qt : 128 * (qt + 1), :], y[:])

    nc.compile()
    return nc


def _numpy_reference(q_, k_, v_, mask, Wq, bq, Wk, bk, Wv, bv, Wf, bf):
    q_ = np.asarray(q_, np.float32)
    k_ = np.asarray(k_, np.float32)
    v_ = np.asarray(v_, np.float32)
    b = q_.shape[0]

    def split(x):
        return x.reshape(b, -1, H, DH).transpose(0, 2, 1, 3)

    q = split(q_ @ Wq + bq)
    k = split(k_ @ Wk + bk)
    v = split(v_ @ Wv + bv)
    attn = np.einsum("bhqd,bhkd->bhqk", q, k) / np.sqrt(np.float32(DH))
    attn = np.where(np.asarray(mask)[:, None, :, None], attn, np.float32(-1e12))
    attn = attn - attn.max(axis=-1, keepdims=True)
    e = np.exp(attn)
    p = e / e.sum(axis=-1, keepdims=True)
    o = np.einsum("bhqk,bhkd->bhqd", p, v)
    o = o.transpose(0, 2, 1, 3).reshape(b, -1, D)
    return (o + q_) @ Wf + bf


# plane-permuted column order for Wq/Wk: [h0 dh0-31, h1 dh0-31, h0 dh32-63, ...]
_PERM = np.concatenate(
    [np.arange(0, 32), np.arange(64, 96), np.arange(32, 64), np.arange(96, 128)]
)


def kernel(q_, k_, v_, mask, Wq, bq, Wk, bk, Wv, bv, Wf, bf):
    mask = np.asarray(mask)
    if not mask.all():
        return _numpy_reference(q_, k_, v_, mask, Wq, bq, Wk, bk, Wv, bv, Wf, bf)

    q_ = np.asarray(q_, np.float32)
    k_ = np.asarray(k_, np.float32)
    v_ = np.asarray(v_, np.float32)
    Wq = np.asarray(Wq, np.float32)
    Wk = np.asarray(Wk, np.float32)
    Wv = np.asarray(Wv, np.float32)
    Wf = np.asarray(Wf, np.float32)
    bq = np.asarray(bq, np.float32)
    bk = np.asarray(bk, np.float32)
    bv = np.asarray(bv, np.float32)

    xq_c = np.ascontiguousarray(np.concatenate([q_[b].T for b in range(B)], axis=1))
    xq8 = xq_c.astype(FP8NP)
    xk8 = np.ascontiguousarray(
        np.concatenate([k_[b].T for b in range(B)], axis=1)
    ).astype(FP8NP)
    xv8 = np.ascontiguousarray(
        np.concatenate([v_[b].T for b in range(B)], axis=1)
    ).astype(FP8NP)
    wf_b = np.ascontiguousarray(Wf).astype(BF16NP)
    wf8_b = np.ascontiguousarray(Wf * W2).astype(FP8NP)

    in_maps = []
    for c in range(N_CORES):
        d0 = 128 * c
        perm = d0 + _PERM
        # per-plane biases: partition p of plane pl holds dout perm[64*pl + p]
        bqk = np.stack(
            [
                bq[perm[0:64]] * WSCALE,
                bq[perm[64:128]] * WSCALE,
                bk[perm[0:64]] * WSCALE,
                bk[perm[64:128]] * WSCALE,
            ],
            axis=1,
        )
        in_maps.append(
            {
                "xq": xq8,
                "xk": xk8,
                "xv": xv8,
                "wq": np.ascontiguousarray(Wq[:, perm] * WSCALE).astype(FP8NP),
                "wk": np.ascontiguousarray(Wk[:, perm] * WSCALE).astype(FP8NP),
                "wv": np.ascontiguousarray(
                    Wv[:, d0 : d0 + 128] * WSCALE
                ).astype(FP8NP),
                "bqk": np.ascontiguousarray(bqk),
                "bv": np.ascontiguousarray(
                    bv[None, d0 : d0 + 128] * WSCALE
                ).astype(BF16NP),
                "wf": wf_b,
                "wf8": wf8_b,
                "xres": np.ascontiguousarray(
                    q_[c // 4].T[:, 512 * (c % 4) : 512 * (c % 4 + 1)]
                ).astype(BF16NP),
            }
        )

    if "nc" not in _CACHE:
        _CACHE["nc"] = _build()
    res = run_bass_kernel_spmd(_CACHE["nc"], in_maps, core_ids=list(range(N_CORES)))

    out = np.empty((B, S, D), np.float32)
    for c in range(N_CORES):
        y = res.results[c]["out"]
        out[c // 4, 512 * (c % 4) : 512 * (c % 4 + 1), :] = y
    out += np.asarray(bf, np.float32)[None, None, :]
    return out


if __name__ == "__main__":
    rng = np.random.default_rng(0)
    args = dict(
        q_=rng.standard_normal((B, S, D), dtype=np.float32),
        k_=rng.standard_normal((B, S, D), dtype=np.float32),
        v_=rng.standard_normal((B, S, D), dtype=np.float32),
        mask=np.ones((B, S), bool),
        Wq=rng.standard_normal((D, D), dtype=np.float32) * 0.02,
        bq=np.zeros(D, np.float32),
        Wk=rng.standard_normal((D, D), dtype=np.float32) * 0.02,
        bk=np.zeros(D, np.float32),
        Wv=rng.standard_normal((D, D), dtype=np.float32) * 0.02,
        bv=np.zeros(D, np.float32),
        Wf=rng.standard_normal((D, D), dtype=np.float32) * 0.02,
        bf=np.zeros(D, np.float32),
    )
    got = kernel(**args)
    want = _numpy_reference(**args)
    rel = np.abs(got - want).max() / np.abs(want).max()
    print("rel_err:", rel)


# revision 15
# speedup vs baseline: 1.0243x; 1.0205x over previous
"""Distributed multi-head attention block on 8 TRN2 NeuronCores (v3).

Reference computation (B=2, S=2048, D=1024, H=16, DH=64):
    q = split_heads(q_ @ Wq + bq); k = ...; v = ...
    attn = softmax(q k^T / 8)  (mask is all-ones -> identity row mask)
    out = (merge_heads(attn @ v) + q_) @ Wf + bf

Sharding: 16 heads split 8 ways (2 heads / core); each core handles BOTH
batches.  Virtual q axis is b-major: vq = b*2048 + s (4096 total).

Engine plan (per core c, heads 2c/2c+1, d-slice 128c..128c+128):
  - Q/K/V projections and the attention-half fc run fp8e4m3, DoubleRow
    where the weight load hides under the moving stream.
  - QK^T runs plain fp8 (64-partition contraction, 512-wide stream) --
    DoubleRow would double the stationary load for no stream gain.
  - V carries a ones column -> PV (fp8 DoubleRow over k-tile pairs) also
    emits the softmax denominator row for free.
  - exp splits between ScalarE (true Exp -> fp8) and VectorE (int8
    exponent-bit trick bitcast to fp8e4m3); softmax renormalization
    absorbs the approximation error.
  - normalize: ScalarE drains OT psum -> SBUF f32, VectorE
    reciprocal_approx_fast on the denominator row, GpSimd broadcasts it
    and multiplies -> fp8 ZT.
  - One 8-core AllToAll (fp8, 512KB) exchanges q-chunks; the residual
    half of the fc (xres^T @ Wf, bf16) runs during the A2A window; the
    attention half (zfull^T @ Wf8) is fp8 DoubleRow after it.

Host: casts/transposes inputs, feeds per-core shards, places each
core's [512, 1024] output chunk, adds bf.  Non-all-ones mask falls back
to a numpy reference (never happens with this generator).
"""

import sys

sys.path.insert(0, "/opt/trn_rl_repo")

import ml_dtypes
import numpy as np

import concourse.bass as bass
import concourse.tile as tile
from concourse import bacc, mybir
from concourse.bass_utils import run_bass_kernel_spmd

B, S, D, H = 2, 2048, 1024, 16
DH = D // H  # 64
N_CORES = 8
VQ = B * S  # 4096
NQC = VQ // 512  # 8 q-chunks
NKT = S // 128  # 16 k-tiles per batch
NPAIR = NKT // 2  # 8 k-tile pairs
NDIN = D // 128  # 8 din tiles

BF16 = mybir.dt.bfloat16
FP8 = mybir.dt.float8e4
I8 = mybir.dt.int8
F32 = mybir.dt.float32
AF = mybir.ActivationFunctionType
ALU = mybir.AluOpType
DR = mybir.MatmulPerfMode.DoubleRow
BF16NP = ml_dtypes.bfloat16
FP8NP = ml_dtypes.float8_e4m3

WSCALE = 32.0  # fp8 weight scale for q/k/v projections
W2 = 32.0  # fp8 scale for Wf in the attention-half fc
CEXP = 1.0 / (8.0 * WSCALE * WSCALE)  # score scale folded into exp
LOG2E = 1.4426950408889634
EXP_C1 = float(CEXP * 8.0 * LOG2E)  # DVE bit-trick multiplier
EXP_C2 = 56.0  # e4m3 exponent bias * 8

# kt indices handled by ScalarE's true exp; the rest use the DVE trick.
ACT_KT = set(kt for kt in range(NKT) if kt % 2 == 0)

_CACHE = {}


def _build():
    nc = bacc.Bacc(None, target_bir_lowering=False)

    xq = nc.declare_dram_parameter("xq", [D, VQ], FP8, isOutput=False)
    xk = nc.declare_dram_parameter("xk", [D, VQ], FP8, isOutput=False)
    xv = nc.declare_dram_parameter("xv", [D, VQ], FP8, isOutput=False)
    wq = nc.declare_dram_parameter("wq", [D, 128], FP8, isOutput=False)
    wk = nc.declare_dram_parameter("wk", [D, 128], FP8, isOutput=False)
    wv = nc.declare_dram_parameter("wv", [D, 128], FP8, isOutput=False)
    bqk = nc.declare_dram_parameter("bqk", [128, 2], F32, isOutput=False)
    bv = nc.declare_dram_parameter("bv", [1, 128], BF16, isOutput=False)
    wf = nc.declare_dram_parameter("wf", [D, D], BF16, isOutput=False)
    wf8 = nc.declare_dram_parameter("wf8", [D, D], FP8, isOutput=False)
    xres = nc.declare_dram_parameter("xres", [D, 512], BF16, isOutput=False)
    out = nc.declare_dram_parameter("out", [512, D], F32, isOutput=True)

    with tile.TileContext(nc) as tc:
        with (
            tc.tile_pool(name="persist", bufs=1) as sbp,
            tc.tile_pool(name="dram", bufs=1, space="DRAM") as dram,
        ):
            # ---- persistent SBUF ----
            qt_sb = sbp.tile([128, VQ], FP8)  # [2h x 64dh, vq]
            kt_sb = sbp.tile([128, VQ], FP8)
            v8 = sbp.tile([128, 2 * NPAIR, 2, 160], FP8)  # (b,pair), sub, 2x80
            wq_sb = sbp.tile([128, NDIN, 128], FP8)
            wk_sb = sbp.tile([128, NDIN, 128], FP8)
            wv_sb = sbp.tile([128, NDIN, 128], FP8)
            bqk_sb = sbp.tile([128, 2], F32)
            bv_sb = sbp.tile([1, 128], BF16)
            bvb_sb = sbp.tile([128, 128], BF16)  # bv broadcast across partitions
            ones_bf = sbp.tile([1, 128], BF16)
            nc.vector.memset(ones_bf[:], 1.0)
            # rowsum ones columns: offset 64 within each 80-wide head slot
            nc.vector.memset(
                v8[:].rearrange("p a s (h n) -> p (a s h) n", n=80)[:, :, 64:65], 1.0
            )

            a2a_in = dram.tile([1024, 512], FP8)
            a2a_out = dram.tile([1024, 512], FP8)

            # ---- input DMAs ----
            # scalar queue: small weights needed immediately
            nc.scalar.dma_start(wk_sb[:], wk[:].rearrange("(t p) n -> p t n", p=128))
            nc.scalar.dma_start(wv_sb[:], wv[:].rearrange("(t p) n -> p t n", p=128))
            nc.scalar.dma_start(wq_sb[:], wq[:].rearrange("(t p) n -> p t n", p=128))
            nc.scalar.dma_start(bqk_sb[:], bqk[:])
            nc.scalar.dma_start(bv_sb[:], bv[:])
            nc.gpsimd.partition_broadcast(bvb_sb[:], bv_sb[:])

            # sync queue: big x streams, k/v-first so attention starts early
            xst = tc.alloc_tile_pool(name="xst", bufs=2)
            xk_b, xv_b, xq_b = [], [], []
            for b in range(B):
                for nm, lst in (("xk", xk_b), ("xv", xv_b), ("xq", xq_b)):
                    lst.append(
                        xst.tile([128, NDIN, 2048], FP8, name=f"{nm}{b}", tag=nm)
                    )
            for b in range(B):
                for lst, src in ((xk_b, xk), (xv_b, xv), (xq_b, xq)):
                    nc.sync.dma_start(
                        lst[b][:],
                        src[:, 2048 * b : 2048 * (b + 1)].rearrange(
                            "(t p) v -> p t v", p=128
                        ),
                    )
            # fc-stage tensors ride the sync queue after the x streams
            wf_sb = sbp.tile([128, NDIN, 1024], BF16)
            wf8_sb = sbp.tile([128, NDIN, 1024], FP8)
            xres_sb = sbp.tile([128, NDIN, 512], BF16)
            yres_sb = sbp.tile([128, 4, 1024], BF16)
            zf_sb = sbp.tile([128, NDIN, 512], FP8)
            nc.sync.dma_start(xres_sb[:], xres[:].rearrange("(t p) n -> p t n", p=128))
            nc.sync.dma_start(wf_sb[:], wf[:].rearrange("(t p) n -> p t n", p=128))
            nc.sync.dma_start(wf8_sb[:], wf8[:].rearrange("(t p) n -> p t n", p=128))

            # ---- projection helpers ----
            def qk_proj_emit(qkps, dst, w_sb, bcol, xt, b, win, copy_engine):
                """One 512-q window of the Q or K projection (fp8 DoubleRow)."""
                q0 = 2048 * b + 512 * win
                ps = qkps.tile([128, 512], F32, name=f"qkp{b}_{win}", tag="qkps")
                for dp in range(4):
                    nc.tensor.matmul(
                        ps[:],
                        lhsT=w_sb[:, 2 * dp : 2 * dp + 2, :],
                        rhs=xt[:, 2 * dp : 2 * dp + 2, 512 * win : 512 * win + 512],
                        start=(dp == 0),
                        stop=(dp == 3),
                        perf_mode=DR,
                    )
                if copy_engine == "act":
                    nc.scalar.activation(
                        dst[:, q0 : q0 + 512],
                        ps[:],
                        AF.Identity,
                        bias=bqk_sb[:, bcol : bcol + 1],
                    )
                else:
                    nc.vector.tensor_scalar_add(
                        dst[:, q0 : q0 + 512], ps[:], bqk_sb[:, bcol : bcol + 1]
                    )

            def v_unit(vps, b, kt):
                vp = vps.tile([128, 128], F32, name=f"vp{b}_{kt}", tag="vps")
                for dp in range(4):
                    nc.tensor.matmul(
                        vp[:],
                        lhsT=xv_b[b][:, 2 * dp : 2 * dp + 2, 128 * kt : 128 * (kt + 1)],
                        rhs=wv_sb[:, 2 * dp : 2 * dp + 2, :],
                        start=(dp == 0),
                        stop=(dp == 3),
                        perf_mode=DR,
                    )
                slot = NPAIR * b + kt // 2
                nc.vector.tensor_tensor(
                    v8[:, slot, kt % 2, :].rearrange("p (h n) -> p h n", h=2)[
                        :, :, 0:64
                    ],
                    vp[:].rearrange("p (h n) -> p h n", n=64),
                    bvb_sb[:].rearrange("p (h n) -> p h n", n=64),
                    ALU.add,
                )

            # ================= phase 1: batch-0 projections =================
            with (
                tc.tile_pool(name="qkps1", bufs=2, space="PSUM") as qkps1,
                tc.tile_pool(name="vps1", bufs=2, space="PSUM") as vps1,
            ):
                warm = vps1.tile([64, 64], F32, name="warm", tag="warm")
                for _ in range(100):
                    nc.tensor.matmul(
                        warm[:], lhsT=ones_bf[0:1, 0:64], rhs=ones_bf[0:1, 0:64],
                        start=True, stop=True,
                    )
                for win in range(4):
                    qk_proj_emit(qkps1, kt_sb, wk_sb, 1, xk_b[0], 0, win, "act")
                for kt in range(NKT):
                    v_unit(vps1, 0, kt)
                for win in range(4):
                    qk_proj_emit(qkps1, qt_sb, wq_sb, 0, xq_b[0], 0, win, "act")

            # ================= phase 2: attention =================
            with (
                tc.tile_pool(name="stp", bufs=2, space="PSUM") as stp,
                tc.tile_pool(name="ptp", bufs=9) as ptp,
                tc.tile_pool(name="nrm", bufs=2) as nrm,
                tc.tile_pool(name="ztp", bufs=4) as ztp,
            ):
                def emit_qk(qc, kt, pt):
                    """Scores for (qc, kt) -> exp into pt[:, kt%2, :]."""
                    b = qc // 4
                    q0 = 512 * qc
                    kk = 2048 * b + 128 * kt
                    st = stp.tile([128, 1024], F32, name=f"st{qc}_{kt}", tag="st")
                    for h in range(2):
                        nc.tensor.matmul(
                            st[:, 512 * h : 512 * (h + 1)],
                            lhsT=kt_sb[64 * h : 64 * (h + 1), kk : kk + 128],
                            rhs=qt_sb[64 * h : 64 * (h + 1), q0 : q0 + 512],
                            start=True,
                            stop=True,
                        )
                    if kt in ACT_KT:
                        nc.scalar.activation(
                            pt[:, kt % 2, :], st[:], AF.Exp, scale=CEXP
                        )
                    else:
                        nc.vector.tensor_scalar(
                            pt[:, kt % 2, :].bitcast(I8),
                            st[:],
                            EXP_C1,
                            EXP_C2,
                            ALU.mult,
                            ALU.add,
                        )

                def emit_qk_pair(qc, pair):
                    pt = ptp.tile([128, 2, 1024], FP8, name=f"pt{qc}_{pair}", tag="pt")
                    emit_qk(qc, 2 * pair, pt)
                    emit_qk(qc, 2 * pair + 1, pt)
                    return pt

                def emit_pv(qc, pair, pt, ot0, ot1):
                    b = qc // 4
                    slot = NPAIR * b + pair
                    for h, ot in ((0, ot0), (1, ot1)):
                        nc.tensor.matmul(
                            ot[:],
                            lhsT=v8[:, slot, :, 80 * h : 80 * h + 65],
                            rhs=pt[:, :, 512 * h : 512 * (h + 1)],
                            start=(pair == 0),
                            stop=(pair == NPAIR - 1),
                            perf_mode=DR,
                        )

                def normalize(qc, ot0, ot1):
                    osb = nrm.tile([65, 1024], F32, name=f"osb{qc}", tag="osb")
                    nc.scalar.copy(osb[:, 0:512], ot0[:])
                    nc.scalar.copy(osb[:, 512:1024], ot1[:])
                    rr = nrm.tile([1, 1024], F32, name=f"rr{qc}", tag="rr")
                    nc.vector.reciprocal(rr[:], osb[64:65, :])
                    zbc = nrm.tile([64, 1024], F32, name=f"zbc{qc}", tag="zbc")
                    nc.gpsimd.partition_broadcast(zbc[:], rr[:])
                    for h in range(2):
                        zt = ztp.tile([64, 512], FP8, name=f"zt{qc}_{h}", tag="zt")
                        nc.gpsimd.tensor_tensor(
                            zt[:],
                            osb[0:64, 512 * h : 512 * (h + 1)],
                            zbc[:, 512 * h : 512 * (h + 1)],
                            ALU.mult,
                        )
                        nc.gpsimd.dma_start(
                            a2a_in[128 * qc + 64 * h : 128 * qc + 64 * h + 64, :],
                            zt[:],
                        )

                # batch-1 projection units, spread across the early chunks
                b1_units = []
                with (
                    tc.tile_pool(name="qkps2", bufs=1, space="PSUM") as qkps2,
                    tc.tile_pool(name="vps2", bufs=1, space="PSUM") as vps2,
                ):
                    for win in range(4):
                        b1_units.append(
                            lambda w=win: qk_proj_emit(
                                qkps2, kt_sb, wk_sb, 1, xk_b[1], 1, w, "act"
                            )
                        )
                    for kt in range(NKT):
                        b1_units.append(lambda k=kt: v_unit(vps2, 1, k))
                    for win in range(4):
                        b1_units.append(
                            lambda w=win: qk_proj_emit(
                                qkps2, qt_sb, wq_sb, 0, xq_b[1], 1, w, "dve"
                            )
                        )
                    ui = 0

                    # prologue: qc0 scores interleaved with early b1 units
                    pts = []
                    for pair in range(NPAIR):
                        pts.append(emit_qk_pair(0, pair))
                        if ui < 8:
                            b1_units[ui]()
                            ui += 1

                    otp = tc.alloc_tile_pool(name="otp", bufs=1, space="PSUM")
                    pending = None
                    for qc in range(NQC):
                        ot0 = otp.tile([65, 512], F32, name=f"ot0_{qc}", tag="ot0")
                        ot1 = otp.tile([65, 512], F32, name=f"ot1_{qc}", tag="ot1")
                        nxt = []
                        for pair in range(NPAIR):
                            emit_pv(qc, pair, pts[pair], ot0, ot1)
                            if qc + 1 < NQC:
                                nxt.append(emit_qk_pair(qc + 1, pair))
                            if pair == 1 and pending is not None:
                                normalize(*pending)
                                pending = None
                            if ui < len(b1_units):
                                b1_units[ui]()
                                ui += 1
                        if pending is not None:
                            normalize(*pending)
                        pending = (qc, ot0, ot1)
                        pts = nxt
                    normalize(*pending)
                    otp.release()

            xst.release()

            # ================= phase 3: A2A + fc =================
            nc.gpsimd.collective_compute(
                "AllToAll",
                ALU.bypass,
                replica_groups=[list(range(N_CORES))],
                ins=[a2a_in.opt()],
                outs=[a2a_out.opt()],
            )

            # residual half of fc (bf16) rides the A2A window
            with (
                tc.tile_pool(name="fcps", bufs=4, space="PSUM") as fcps,
                tc.tile_pool(name="ysb", bufs=2) as ysb,
            ):
                for qt in range(4):
                    for nb in range(2):
                        yp = fcps.tile([128, 512], F32, name=f"yr{qt}_{nb}", tag="yr")
                        for j in range(NDIN):
                            nc.tensor.matmul(
                                yp[:],
                                lhsT=xres_sb[:, j, 128 * qt : 128 * (qt + 1)],
                                rhs=wf_sb[:, j, 512 * nb : 512 * (nb + 1)],
                                start=(j == 0),
                                stop=(j == NDIN - 1),
                            )
                        nc.vector.tensor_copy(
                            yres_sb[:, qt, 512 * nb : 512 * (nb + 1)], yp[:]
                        )

                nc.sync.dma_start(
                    zf_sb[:], a2a_out[:].rearrange("(t p) v -> p t v", p=128)
                )
                # attention half (fp8 DoubleRow) + residual add + store
                for qt in range(4):
                    y = ysb.tile([128, 1024], F32, name=f"y{qt}", tag="y")
                    for nb in range(2):
                        yp = fcps.tile([128, 512], F32, name=f"ya{qt}_{nb}", tag="ya")
                        for dp in range(4):
                            nc.tensor.matmul(
                                yp[:],
                                lhsT=zf_sb[:, 2 * dp : 2 * dp + 2, 128 * qt : 128 * (qt + 1)],
                                rhs=wf8_sb[:, 2 * dp : 2 * dp + 2, 512 * nb : 512 * (nb + 1)],
                                start=(dp == 0),
                                stop=(dp == 3),
                                perf_mode=DR,
                            )
                        nc.vector.scalar_tensor_tensor(
                            y[:, 512 * nb : 512 * (nb + 1)],
                            yp[:],
                            1.0 / (WSCALE * W2),
                            yres_sb[:, qt, 512 * nb : 512 * (nb + 1)],
                            ALU.mult,
                            ALU.add,
                        )
                    nc.sync.dma_start(out[128 * qt : 128 * (qt + 1), :], y[:])

    nc.compile()
    return nc


def _numpy_reference(q_, k_, v_, mask, Wq, bq, Wk, bk, Wv, bv, Wf, bf):
    q_ = np.asarray(q_, np.float32)
    k_ = np.asarray(k_, np.float32)
    v_ = np.asarray(v_, np.float32)
    b = q_.shape[0]

    def split(x):
        return x.reshape(b, -1, H, DH).transpose(0, 2, 1, 3)

    q = split(q_ @ Wq + bq)
    k = split(k_ @ Wk + bk)
    v = split(v_ @ Wv + bv)
    attn = np.einsum("bhqd,bhkd->bhqk", q, k) / np.sqrt(np.float32(DH))
    attn = np.where(np.asarray(mask)[:, None, :, None], attn, np.float32(-1e12))
    attn = attn - attn.max(axis=-1, keepdims=True)
    e = np.exp(attn)
    p = e / e.sum(axis=-1, keepdims=True)
    o = np.einsum("bhqk,bhkd->bhqd", p, v)
    o = o.transpose(0, 2, 1, 3).reshape(b, -1, D)
    return (o + q_) @ Wf + bf


def kernel(q_, k_, v_, mask, Wq, bq, Wk, bk, Wv, bv, Wf, bf):
    mask = np.asarray(mask)
    if not mask.all():
        return _numpy_reference(q_, k_, v_, mask, Wq, bq, Wk, bk, Wv, bv, Wf, bf)

    q_ = np.asarray(q_, np.float32)
    k_ = np.asarray(k_, np.float32)
    v_ = np.asarray(v_, np.float32)
    Wq = np.asarray(Wq, np.float32)
    Wk = np.asarray(Wk, np.float32)
    Wv = np.asarray(Wv, np.float32)
    Wf = np.asarray(Wf, np.float32)
    bq = np.asarray(bq, np.float32)
    bk = np.asarray(bk, np.float32)
    bv = np.asarray(bv, np.float32)

    xq8 = np.ascontiguousarray(
        np.concatenate([q_[b].T for b in range(B)], axis=1)
    ).astype(FP8NP)
    xk8 = np.ascontiguousarray(
        np.concatenate([k_[b].T for b in range(B)], axis=1)
    ).astype(FP8NP)
    xv8 = np.ascontiguousarray(
        np.concatenate([v_[b].T for b in range(B)], axis=1)
    ).astype(FP8NP)
    wf_b = np.ascontiguousarray(Wf).astype(BF16NP)
    wf8_b = np.ascontiguousarray(Wf * W2).astype(FP8NP)

    in_maps = []
    for c in range(N_CORES):
        d0 = 128 * c
        in_maps.append(
            {
                "xq": xq8,
                "xk": xk8,
                "xv": xv8,
                "wq": np.ascontiguousarray(Wq[:, d0 : d0 + 128] * WSCALE).astype(
                    FP8NP
                ),
                "wk": np.ascontiguousarray(Wk[:, d0 : d0 + 128] * WSCALE).astype(
                    FP8NP
                ),
                "wv": np.ascontiguousarray(Wv[:, d0 : d0 + 128] * WSCALE).astype(
                    FP8NP
                ),
                "bqk": np.ascontiguousarray(
                    np.stack(
                        [bq[d0 : d0 + 128] * WSCALE, bk[d0 : d0 + 128] * WSCALE],
                        axis=1,
                    )
                ),
                "bv": np.ascontiguousarray(
                    bv[None, d0 : d0 + 128] * WSCALE
                ).astype(BF16NP),
                "wf": wf_b,
                "wf8": wf8_b,
                "xres": np.ascontiguousarray(
                    q_[c // 4].T[:, 512 * (c % 4) : 512 * (c % 4 + 1)]
                ).astype(BF16NP),
            }
        )

    if "nc" not in _CACHE:
        _CACHE["nc"] = _build()
    res = run_bass_kernel_spmd(_CACHE["nc"], in_maps, core_ids=list(range(N_CORES)))

    out = np.empty((B, S, D), np.float32)
    for c in range(N_CORES):
        y = res.results[c]["out"]
        out[c // 4, 512 * (c % 4) : 512 * (c % 4 + 1), :] = y
    out += np.asarray(bf, np.float32)[None, None, :]
    return out


if __name__ == "__main__":
    rng = np.random.default_rng(0)
    args = dict(
        q_=rng.standard_normal((B, S, D), dtype=np.float32),
        k_=rng.standard_normal((B, S, D), dtype=np.float32),
        v_=rng.standard_normal((B, S, D), dtype=np.float32),
        mask=np.ones((B, S), bool),
        Wq=rng.standard_normal((D, D), dtype=np.float32) * 0.02,
        bq=np.zeros(D, np.float32),
        Wk=rng.standard_normal((D, D), dtype=np.float32) * 0.02,
        bk=np.zeros(D, np.float32),
        Wv=rng.standard_normal((D, D), dtype=np.float32) * 0.02,
        bv=np.zeros(D, np.float32),
        Wf=rng.standard_normal((D, D), dtype=np.float32) * 0.02,
        bf=np.zeros(D, np.float32),
    )
    got = kernel(**args)
    want = _numpy_reference(**args)
    rel = np.abs(got - want).max() / np.abs(want).max()
    print("rel_err:", rel)


# revision 22
# speedup vs baseline: 1.0363x; 1.0117x over previous
"""Distributed multi-head attention block on 8 TRN2 NeuronCores (v3).

Reference computation (B=2, S=2048, D=1024, H=16, DH=64):
    q = split_heads(q_ @ Wq + bq); k = ...; v = ...
    attn = softmax(q k^T / 8)  (mask is all-ones -> identity row mask)
    out = (merge_heads(attn @ v) + q_) @ Wf + bf

Sharding: 16 heads split 8 ways (2 heads / core); each core handles BOTH
batches.  Virtual q axis is b-major: vq = b*2048 + s (4096 total).

Engine plan (per core c, heads 2c/2c+1, d-slice 128c..128c+128):
  - Q/K/V projections and the attention-half fc run fp8e4m3, DoubleRow
    where the weight load hides under the moving stream.
  - QK^T runs plain fp8 (64-partition contraction, 512-wide stream) --
    DoubleRow would double the stationary load for no stream gain.
  - V carries a ones column -> PV (fp8 DoubleRow over k-tile pairs) also
    emits the softmax denominator row for free.
  - exp splits between ScalarE (true Exp -> fp8) and VectorE (int8
    exponent-bit trick bitcast to fp8e4m3); softmax renormalization
    absorbs the approximation error.
  - normalize: ScalarE drains OT psum -> SBUF f32, VectorE
    reciprocal_approx_fast on the denominator row, GpSimd broadcasts it
    and multiplies -> fp8 ZT.
  - One 8-core AllToAll (fp8, 512KB) exchanges q-chunks; the residual
    half of the fc (xres^T @ Wf, bf16) runs during the A2A window; the
    attention half (zfull^T @ Wf8) is fp8 DoubleRow after it.

Host: casts/transposes inputs, feeds per-core shards, places each
core's [512, 1024] output chunk, adds bf.  Non-all-ones mask falls back
to a numpy reference (never happens with this generator).
"""

import sys

sys.path.insert(0, "/opt/trn_rl_repo")

import ml_dtypes
import numpy as np

import concourse.bass as bass
import concourse.tile as tile
from concourse import bacc, mybir
from concourse.bass_utils import run_bass_kernel_spmd

B, S, D, H = 2, 2048, 1024, 16
DH = D // H  # 64
N_CORES = 8
VQ = B * S  # 4096
NQC = VQ // 512  # 8 q-chunks
NKT = S // 128  # 16 k-tiles per batch
NPAIR = NKT // 2  # 8 k-tile pairs
NDIN = D // 128  # 8 din tiles

BF16 = mybir.dt.bfloat16
FP8 = mybir.dt.float8e4
I8 = mybir.dt.int8
F32 = mybir.dt.float32
AF = mybir.ActivationFunctionType
ALU = mybir.AluOpType
DR = mybir.MatmulPerfMode.DoubleRow
BF16NP = ml_dtypes.bfloat16
FP8NP = ml_dtypes.float8_e4m3

WSCALE = 32.0  # fp8 weight scale for q/k/v projections
W2 = 32.0  # fp8 scale for Wf in the attention-half fc
CEXP = 1.0 / (8.0 * WSCALE * WSCALE)  # score scale folded into exp
LOG2E = 1.4426950408889634
EXP_C1 = float(CEXP * 8.0 * LOG2E)  # DVE bit-trick multiplier
EXP_C2 = 56.0  # e4m3 exponent bias * 8

# kt indices handled by ScalarE's true exp; the rest use the DVE trick.
ACT_KT = set(kt for kt in range(NKT) if kt % 2 == 0)

_CACHE = {}


def _build():
    nc = bacc.Bacc(None, target_bir_lowering=False)

    xq = nc.declare_dram_parameter("xq", [D, VQ], FP8, isOutput=False)
    xk = nc.declare_dram_parameter("xk", [D, VQ], FP8, isOutput=False)
    xv = nc.declare_dram_parameter("xv", [D, VQ], FP8, isOutput=False)
    wq = nc.declare_dram_parameter("wq", [128, NDIN * 128], FP8, isOutput=False)
    wk = nc.declare_dram_parameter("wk", [128, NDIN * 128], FP8, isOutput=False)
    wv = nc.declare_dram_parameter("wv", [128, NDIN * 128], FP8, isOutput=False)
    bqk = nc.declare_dram_parameter("bqk", [128, 2], F32, isOutput=False)
    bv = nc.declare_dram_parameter("bv", [1, 128], BF16, isOutput=False)
    wf = nc.declare_dram_parameter("wf", [128, NDIN * 1024], BF16, isOutput=False)
    wf8 = nc.declare_dram_parameter("wf8", [128, NDIN * 1024], FP8, isOutput=False)
    xres = nc.declare_dram_parameter("xres", [128, NDIN * 512], BF16, isOutput=False)
    out = nc.declare_dram_parameter("out", [512, D], F32, isOutput=True)

    with tile.TileContext(nc) as tc:
        with (
            tc.tile_pool(name="persist", bufs=1) as sbp,
            tc.tile_pool(name="dram", bufs=1, space="DRAM") as dram,
        ):
            # ---- persistent SBUF ----
            qt_sb = sbp.tile([128, VQ], FP8)  # [2h x 64dh, vq]
            kt_sb = sbp.tile([128, VQ], FP8)
            v8 = sbp.tile([128, 2 * NPAIR, 2, 160], FP8)  # (b,pair), sub, 2x80
            wq_sb = sbp.tile([128, NDIN, 128], FP8)
            wk_sb = sbp.tile([128, NDIN, 128], FP8)
            wv_sb = sbp.tile([128, NDIN, 128], FP8)
            bqk_sb = sbp.tile([128, 2], F32)
            bv_sb = sbp.tile([1, 128], BF16)
            bvb_sb = sbp.tile([128, 128], BF16)  # bv broadcast across partitions
            ones_bf = sbp.tile([1, 128], BF16)
            nc.vector.memset(ones_bf[:], 1.0)
            # rowsum ones columns: offset 64 within each 80-wide head slot
            nc.vector.memset(
                v8[:].rearrange("p a s (h n) -> p (a s h) n", n=80)[:, :, 64:65], 1.0
            )

            a2a_in = dram.tile([1024, 512], FP8)
            a2a_out = dram.tile([1024, 512], FP8)

            # ---- input DMAs ----
            # scalar queue: small weights needed immediately
            nc.scalar.dma_start(wk_sb[:], wk[:].rearrange("p (t n) -> p t n", t=NDIN))
            nc.scalar.dma_start(wv_sb[:], wv[:].rearrange("p (t n) -> p t n", t=NDIN))
            nc.scalar.dma_start(wq_sb[:], wq[:].rearrange("p (t n) -> p t n", t=NDIN))
            nc.scalar.dma_start(bqk_sb[:], bqk[:])
            nc.scalar.dma_start(bv_sb[:], bv[:])
            nc.gpsimd.partition_broadcast(bvb_sb[:], bv_sb[:])

            # sync queue: big x streams, k/v-first so attention starts early
            xst = tc.alloc_tile_pool(name="xst", bufs=2)
            xk_b, xv_b, xq_b = [], [], []
            for b in range(B):
                for nm, lst in (("xk", xk_b), ("xv", xv_b), ("xq", xq_b)):
                    lst.append(
                        xst.tile([128, NDIN, 2048], FP8, name=f"{nm}{b}", tag=nm)
                    )
            for b, eng in ((0, nc.sync), (1, nc.gpsimd)):
                for lst, srct in ((xk_b, xk), (xv_b, xv), (xq_b, xq)):
                    eng.dma_start(
                        lst[b][:],
                        srct[:, 2048 * b : 2048 * (b + 1)].rearrange(
                            "(t p) v -> p t v", p=128
                        ),
                    )

            # ---- projection helpers ----
            def qk_proj_emit(qkps, dst, w_sb, bcol, xt, b, win, copy_engine):
                """One 512-q window of the Q or K projection (fp8 DoubleRow)."""
                q0 = 2048 * b + 512 * win
                ps = qkps.tile([128, 512], F32, name=f"qkp{b}_{win}", tag="qkps")
                for dp in range(4):
                    nc.tensor.matmul(
                        ps[:],
                        lhsT=w_sb[:, 2 * dp : 2 * dp + 2, :],
                        rhs=xt[:, 2 * dp : 2 * dp + 2, 512 * win : 512 * win + 512],
                        start=(dp == 0),
                        stop=(dp == 3),
                        perf_mode=DR,
                    )
                if copy_engine == "act":
                    nc.scalar.activation(
                        dst[:, q0 : q0 + 512],
                        ps[:],
                        AF.Identity,
                        bias=bqk_sb[:, bcol : bcol + 1],
                    )
                else:
                    nc.vector.tensor_scalar_add(
                        dst[:, q0 : q0 + 512], ps[:], bqk_sb[:, bcol : bcol + 1]
                    )

            def v_unit(vps, b, kt):
                vp = vps.tile([128, 128], F32, name=f"vp{b}_{kt}", tag="vps")
                for dp in range(4):
                    nc.tensor.matmul(
                        vp[:],
                        lhsT=xv_b[b][:, 2 * dp : 2 * dp + 2, 128 * kt : 128 * (kt + 1)],
                        rhs=wv_sb[:, 2 * dp : 2 * dp + 2, :],
                        start=(dp == 0),
                        stop=(dp == 3),
                        perf_mode=DR,
                    )
                slot = NPAIR * b + kt // 2
                nc.vector.tensor_tensor(
                    v8[:, slot, kt % 2, :].rearrange("p (h n) -> p h n", h=2)[
                        :, :, 0:64
                    ],
                    vp[:].rearrange("p (h n) -> p h n", n=64),
                    bvb_sb[:].rearrange("p (h n) -> p h n", n=64),
                    ALU.add,
                )

            # ================= phase 1: batch-0 projections =================
            with (
                tc.tile_pool(name="qkps1", bufs=2, space="PSUM") as qkps1,
                tc.tile_pool(name="vps1", bufs=2, space="PSUM") as vps1,
            ):
                warm = vps1.tile([64, 64], F32, name="warm", tag="warm")
                for _ in range(100):
                    nc.tensor.matmul(
                        warm[:], lhsT=ones_bf[0:1, 0:64], rhs=ones_bf[0:1, 0:64],
                        start=True, stop=True,
                    )
                for win in range(4):
                    qk_proj_emit(qkps1, kt_sb, wk_sb, 1, xk_b[0], 0, win, "act")
                for kt in range(NKT):
                    v_unit(vps1, 0, kt)
                for win in range(4):
                    qk_proj_emit(qkps1, qt_sb, wq_sb, 0, xq_b[0], 0, win, "act")

            # ================= phase 2: attention =================
            with (
                tc.tile_pool(name="stp", bufs=2, space="PSUM") as stp,
                tc.tile_pool(name="ptp", bufs=9) as ptp,
                tc.tile_pool(name="nrm", bufs=2) as nrm,
                tc.tile_pool(name="ztp", bufs=4) as ztp,
            ):
                def emit_qk(qc, kt, pt):
                    """Scores for (qc, kt) -> exp into pt[:, kt%2, :]."""
                    b = qc // 4
                    q0 = 512 * qc
                    kk = 2048 * b + 128 * kt
                    st = stp.tile([128, 1024], F32, name=f"st{qc}_{kt}", tag="st")
                    for h in range(2):
                        nc.tensor.matmul(
                            st[:, 512 * h : 512 * (h + 1)],
                            lhsT=kt_sb[64 * h : 64 * (h + 1), kk : kk + 128],
                            rhs=qt_sb[64 * h : 64 * (h + 1), q0 : q0 + 512],
                            start=True,
                            stop=True,
                        )
                    if kt in ACT_KT:
                        nc.scalar.activation(
                            pt[:, kt % 2, :], st[:], AF.Exp, scale=CEXP
                        )
                    else:
                        nc.vector.tensor_scalar(
                            pt[:, kt % 2, :].bitcast(I8),
                            st[:],
                            EXP_C1,
                            EXP_C2,
                            ALU.mult,
                            ALU.add,
                        )

                def emit_qk_pair(qc, pair):
                    pt = ptp.tile([128, 2, 1024], FP8, name=f"pt{qc}_{pair}", tag="pt")
                    emit_qk(qc, 2 * pair, pt)
                    emit_qk(qc, 2 * pair + 1, pt)
                    return pt

                def emit_pv(qc, pair, pt, ot0, ot1):
                    b = qc // 4
                    slot = NPAIR * b + pair
                    for h, ot in ((0, ot0), (1, ot1)):
                        nc.tensor.matmul(
                            ot[:],
                            lhsT=v8[:, slot, :, 80 * h : 80 * h + 65],
                            rhs=pt[:, :, 512 * h : 512 * (h + 1)],
                            start=(pair == 0),
                            stop=(pair == NPAIR - 1),
                            perf_mode=DR,
                        )

                def normalize(qc, ot0, ot1):
                    osb = nrm.tile([64, 1024], F32, name=f"osb{qc}", tag="osb")
                    rsf = nrm.tile([1, 1024], F32, name=f"rsf{qc}", tag="rsf")
                    for h, ot in ((0, ot0), (1, ot1)):
                        nc.scalar.copy(osb[:, 512 * h : 512 * (h + 1)], ot[0:64, :])
                        nc.scalar.copy(rsf[:, 512 * h : 512 * (h + 1)], ot[64:65, :])
                    rr = nrm.tile([1, 1024], F32, name=f"rr{qc}", tag="rr")
                    nc.vector.reciprocal_approx_fast(rr[:], rsf[:])
                    zbc = nrm.tile([64, 1024], F32, name=f"zbc{qc}", tag="zbc")
                    nc.gpsimd.partition_broadcast(zbc[:], rr[:])
                    for h in range(2):
                        zt = ztp.tile([64, 512], FP8, name=f"zt{qc}_{h}", tag="zt")
                        nc.gpsimd.tensor_tensor(
                            zt[:],
                            osb[:, 512 * h : 512 * (h + 1)],
                            zbc[:, 512 * h : 512 * (h + 1)],
                            ALU.mult,
                        )
                        nc.gpsimd.dma_start(
                            a2a_in[128 * qc + 64 * h : 128 * qc + 64 * h + 64, :],
                            zt[:],
                        )

                # batch-1 projection units, spread across the early chunks
                b1_units = []
                with (
                    tc.tile_pool(name="qkps2", bufs=1, space="PSUM") as qkps2,
                    tc.tile_pool(name="vps2", bufs=1, space="PSUM") as vps2,
                ):
                    for win in range(4):
                        b1_units.append(
                            lambda w=win: qk_proj_emit(
                                qkps2, kt_sb, wk_sb, 1, xk_b[1], 1, w, "act"
                            )
                        )
                    for kt in range(NKT):
                        b1_units.append(lambda k=kt: v_unit(vps2, 1, k))
                    for win in range(4):
                        b1_units.append(
                            lambda w=win: qk_proj_emit(
                                qkps2, qt_sb, wq_sb, 0, xq_b[1], 1, w, "dve"
                            )
                        )
                    ui = 0

                    # prologue: qc0 scores interleaved with early b1 units
                    pts = []
                    for pair in range(NPAIR):
                        pts.append(emit_qk_pair(0, pair))
                        if ui < 8:
                            b1_units[ui]()
                            ui += 1

                    otp = tc.alloc_tile_pool(name="otp", bufs=1, space="PSUM")
                    pending = None
                    for qc in range(NQC):
                        ot0 = otp.tile([65, 512], F32, name=f"ot0_{qc}", tag="ot0")
                        ot1 = otp.tile([65, 512], F32, name=f"ot1_{qc}", tag="ot1")
                        nxt = []
                        for pair in range(NPAIR):
                            emit_pv(qc, pair, pts[pair], ot0, ot1)
                            if qc + 1 < NQC:
                                nxt.append(emit_qk_pair(qc + 1, pair))
                            if pair == 1 and pending is not None:
                                normalize(*pending)
                                pending = None
                            if ui < len(b1_units):
                                b1_units[ui]()
                                ui += 1
                        if pending is not None:
                            normalize(*pending)
                        pending = (qc, ot0, ot1)
                        pts = nxt
                    normalize(*pending)
                    otp.release()

            xst.release()

            # ================= phase 3: fc-stage tensors + Yres + A2A + fc ==
            late = tc.alloc_tile_pool(name="late", bufs=1)
            wf_sb = late.tile([128, NDIN, 1024], BF16, name="wf_sb")
            wf8_sb = late.tile([128, NDIN, 1024], FP8, name="wf8_sb")
            xres_sb = late.tile([128, NDIN, 512], BF16, name="xres_sb")
            yres_sb = late.tile([128, 4, 1024], BF16, name="yres_sb")
            zf_sb = late.tile([128, NDIN, 512], FP8, name="zf_sb")
            nc.sync.dma_start(xres_sb[:], xres[:].rearrange("p (t n) -> p t n", t=NDIN))
            nc.sync.dma_start(wf_sb[:], wf[:].rearrange("p (t n) -> p t n", t=NDIN))
            nc.sync.dma_start(wf8_sb[:], wf8[:].rearrange("p (t n) -> p t n", t=NDIN))

            with (
                tc.tile_pool(name="fcps", bufs=4, space="PSUM") as fcps,
                tc.tile_pool(name="ysb", bufs=2) as ysb,
            ):
                for qt in range(4):
                    for nb in range(2):
                        yp = fcps.tile([128, 512], F32, name=f"yr{qt}_{nb}", tag="yr")
                        for j in range(NDIN):
                            nc.tensor.matmul(
                                yp[:],
                                lhsT=xres_sb[:, j, 128 * qt : 128 * (qt + 1)],
                                rhs=wf_sb[:, j, 512 * nb : 512 * (nb + 1)],
                                start=(j == 0),
                                stop=(j == NDIN - 1),
                            )
                        nc.vector.tensor_copy(
                            yres_sb[:, qt, 512 * nb : 512 * (nb + 1)], yp[:]
                        )

                nc.gpsimd.collective_compute(
                    "AllToAll",
                    ALU.bypass,
                    replica_groups=[list(range(N_CORES))],
                    ins=[a2a_in.opt()],
                    outs=[a2a_out.opt()],
                )
                nc.sync.dma_start(
                    zf_sb[:], a2a_out[:].rearrange("(t p) v -> p t v", p=128)
                )
                # attention half (fp8 DoubleRow) + residual add + store
                for qt in range(4):
                    y = ysb.tile([128, 1024], F32, name=f"y{qt}", tag="y")
                    for nb in range(2):
                        yp = fcps.tile([128, 512], F32, name=f"ya{qt}_{nb}", tag="ya")
                        for dp in range(4):
                            nc.tensor.matmul(
                                yp[:],
                                lhsT=zf_sb[:, 2 * dp : 2 * dp + 2, 128 * qt : 128 * (qt + 1)],
                                rhs=wf8_sb[:, 2 * dp : 2 * dp + 2, 512 * nb : 512 * (nb + 1)],
                                start=(dp == 0),
                                stop=(dp == 3),
                                perf_mode=DR,
                            )
                        nc.vector.scalar_tensor_tensor(
                            y[:, 512 * nb : 512 * (nb + 1)],
                            yp[:],
                            1.0 / (WSCALE * W2),
                            yres_sb[:, qt, 512 * nb : 512 * (nb + 1)],
                            ALU.mult,
                            ALU.add,
                        )
                    nc.sync.dma_start(out[128 * qt : 128 * (qt + 1), :], y[:])

            late.release()

    nc.compile()
    return nc


def _to_sbuf_layout(w, inner):
    """[D, N] -> [128, NDIN*inner] with w_pre[p, t*inner+n] = w[128t+p, n]."""
    return np.ascontiguousarray(
        w.reshape(NDIN, 128, inner).transpose(1, 0, 2).reshape(128, NDIN * inner)
    )


def _numpy_reference(q_, k_, v_, mask, Wq, bq, Wk, bk, Wv, bv, Wf, bf):
    q_ = np.asarray(q_, np.float32)
    k_ = np.asarray(k_, np.float32)
    v_ = np.asarray(v_, np.float32)
    b = q_.shape[0]

    def split(x):
        return x.reshape(b, -1, H, DH).transpose(0, 2, 1, 3)

    q = split(q_ @ Wq + bq)
    k = split(k_ @ Wk + bk)
    v = split(v_ @ Wv + bv)
    attn = np.einsum("bhqd,bhkd->bhqk", q, k) / np.sqrt(np.float32(DH))
    attn = np.where(np.asarray(mask)[:, None, :, None], attn, np.float32(-1e12))
    attn = attn - attn.max(axis=-1, keepdims=True)
    e = np.exp(attn)
    p = e / e.sum(axis=-1, keepdims=True)
    o = np.einsum("bhqk,bhkd->bhqd", p, v)
    o = o.transpose(0, 2, 1, 3).reshape(b, -1, D)
    return (o + q_) @ Wf + bf


def kernel(q_, k_, v_, mask, Wq, bq, Wk, bk, Wv, bv, Wf, bf):
    mask = np.asarray(mask)
    if not mask.all():
        return _numpy_reference(q_, k_, v_, mask, Wq, bq, Wk, bk, Wv, bv, Wf, bf)

    q_ = np.asarray(q_, np.float32)
    k_ = np.asarray(k_, np.float32)
    v_ = np.asarray(v_, np.float32)
    Wq = np.asarray(Wq, np.float32)
    Wk = np.asarray(Wk, np.float32)
    Wv = np.asarray(Wv, np.float32)
    Wf = np.asarray(Wf, np.float32)
    bq = np.asarray(bq, np.float32)
    bk = np.asarray(bk, np.float32)
    bv = np.asarray(bv, np.float32)

    xq8 = np.ascontiguousarray(
        np.concatenate([q_[b].T for b in range(B)], axis=1)
    ).astype(FP8NP)
    xk8 = np.ascontiguousarray(
        np.concatenate([k_[b].T for b in range(B)], axis=1)
    ).astype(FP8NP)
    xv8 = np.ascontiguousarray(
        np.concatenate([v_[b].T for b in range(B)], axis=1)
    ).astype(FP8NP)
    wf_b = _to_sbuf_layout(Wf, 1024).astype(BF16NP)
    wf8_b = _to_sbuf_layout(Wf * W2, 1024).astype(FP8NP)

    in_maps = []
    for c in range(N_CORES):
        d0 = 128 * c
        in_maps.append(
            {
                "xq": xq8,
                "xk": xk8,
                "xv": xv8,
                "wq": _to_sbuf_layout(Wq[:, d0 : d0 + 128] * WSCALE, 128).astype(
                    FP8NP
                ),
                "wk": _to_sbuf_layout(Wk[:, d0 : d0 + 128] * WSCALE, 128).astype(
                    FP8NP
                ),
                "wv": _to_sbuf_layout(Wv[:, d0 : d0 + 128] * WSCALE, 128).astype(
                    FP8NP
                ),
                "bqk": np.ascontiguousarray(
                    np.stack(
                        [bq[d0 : d0 + 128] * WSCALE, bk[d0 : d0 + 128] * WSCALE],
                        axis=1,
                    )
                ),
                "bv": np.ascontiguousarray(
                    bv[None, d0 : d0 + 128] * WSCALE
                ).astype(BF16NP),
                "wf": wf_b,
                "wf8": wf8_b,
                "xres": _to_sbuf_layout(
                    np.ascontiguousarray(
                        q_[c // 4].T[:, 512 * (c % 4) : 512 * (c % 4 + 1)]
                    ),
                    512,
                ).astype(BF16NP),
            }
        )

    if "nc" not in _CACHE:
        _CACHE["nc"] = _build()
    res = run_bass_kernel_spmd(_CACHE["nc"], in_maps, core_ids=list(range(N_CORES)))

    out = np.empty((B, S, D), np.float32)
    for c in range(N_CORES):
        y = res.results[c]["out"]
        out[c // 4, 512 * (c % 4) : 512 * (c % 4 + 1), :] = y
    out += np.asarray(bf, np.float32)[None, None, :]
    return out


if __name__ == "__main__":
    rng = np.random.default_rng(0)
    args = dict(
        q_=rng.standard_normal((B, S, D), dtype=np.float32),
        k_=rng.standard_normal((B, S, D), dtype=np.float32),
        v_=rng.standard_normal((B, S, D), dtype=np.float32),
        mask=np.ones((B, S), bool),
        Wq=rng.standard_normal((D, D), dtype=np.float32) * 0.02,
        bq=np.zeros(D, np.float32),
        Wk=rng.standard_normal((D, D), dtype=np.float32) * 0.02,
        bk=np.zeros(D, np.float32),
        Wv=rng.standard_normal((D, D), dtype=np.float32) * 0.02,
        bv=np.zeros(D, np.float32),
        Wf=rng.standard_normal((D, D), dtype=np.float32) * 0.02,
        bf=np.zeros(D, np.float32),
    )
    got = kernel(**args)
    want = _numpy_reference(**args)
    rel = np.abs(got - want).max() / np.abs(want).max()
    print("rel_err:", rel)


# revision 26
# speedup vs baseline: 1.1310x; 1.0914x over previous
"""Distributed multi-head attention block on 8 TRN2 NeuronCores (v3).

Reference computation (B=2, S=2048, D=1024, H=16, DH=64):
    q = split_heads(q_ @ Wq + bq); k = ...; v = ...
    attn = softmax(q k^T / 8)  (mask is all-ones -> identity row mask)
    out = (merge_heads(attn @ v) + q_) @ Wf + bf

Sharding: 16 heads split 8 ways (2 heads / core); each core handles BOTH
batches.  Virtual q axis is b-major: vq = b*2048 + s (4096 total).

Engine plan (per core c, heads 2c/2c+1, d-slice 128c..128c+128):
  - Q/K/V projections and the attention-half fc run fp8e4m3, DoubleRow
    where the weight load hides under the moving stream.
  - QK^T runs plain fp8 (64-partition contraction, 512-wide stream) --
    DoubleRow would double the stationary load for no stream gain.
  - V carries a ones column -> PV (fp8 DoubleRow over k-tile pairs) also
    emits the softmax denominator row for free.
  - exp splits between ScalarE (true Exp -> fp8) and VectorE (int8
    exponent-bit trick bitcast to fp8e4m3); softmax renormalization
    absorbs the approximation error.
  - normalize: ScalarE drains OT psum -> SBUF f32, VectorE
    reciprocal_approx_fast on the denominator row, GpSimd broadcasts it
    and multiplies -> fp8 ZT.
  - One 8-core AllToAll (fp8, 512KB) exchanges q-chunks; the residual
    half of the fc (xres^T @ Wf, bf16) runs during the A2A window; the
    attention half (zfull^T @ Wf8) is fp8 DoubleRow after it.

Host: casts/transposes inputs, feeds per-core shards, places each
core's [512, 1024] output chunk, adds bf.  Non-all-ones mask falls back
to a numpy reference (never happens with this generator).
"""

import sys

sys.path.insert(0, "/opt/trn_rl_repo")

import ml_dtypes
import numpy as np

import concourse.bass as bass
import concourse.tile as tile
from concourse import bacc, mybir
from concourse.bass_utils import run_bass_kernel_spmd

B, S, D, H = 2, 2048, 1024, 16
DH = D // H  # 64
N_CORES = 8
VQ = B * S  # 4096
NQC = VQ // 512  # 8 q-chunks
NKT = S // 128  # 16 k-tiles per batch
NPAIR = NKT // 2  # 8 k-tile pairs
NDIN = D // 128  # 8 din tiles

BF16 = mybir.dt.bfloat16
FP8 = mybir.dt.float8e4
I8 = mybir.dt.int8
F32 = mybir.dt.float32
AF = mybir.ActivationFunctionType
ALU = mybir.AluOpType
DR = mybir.MatmulPerfMode.DoubleRow
BF16NP = ml_dtypes.bfloat16
FP8NP = ml_dtypes.float8_e4m3

WSCALE = 32.0  # fp8 weight scale for q/k/v projections
W2 = 32.0  # fp8 scale for Wf in the attention-half fc
CEXP = 1.0 / (8.0 * WSCALE * WSCALE)  # score scale folded into exp
LOG2E = 1.4426950408889634
EXP_C1 = float(CEXP * 8.0 * LOG2E)  # DVE bit-trick multiplier
EXP_C2 = 56.0  # e4m3 exponent bias * 8

# kt indices handled by ScalarE's true exp; the rest use the DVE trick.
ACT_KT = set(kt for kt in range(NKT) if kt % 2 == 0) | {1}

_CACHE = {}


def _build():
    nc = bacc.Bacc(None, target_bir_lowering=False)

    xq = nc.declare_dram_parameter("xq", [D, VQ], FP8, isOutput=False)
    xk = nc.declare_dram_parameter("xk", [D, VQ], FP8, isOutput=False)
    xv = nc.declare_dram_parameter("xv", [D, VQ], FP8, isOutput=False)
    wq = nc.declare_dram_parameter("wq", [128, NDIN * 128], FP8, isOutput=False)
    wk = nc.declare_dram_parameter("wk", [128, NDIN * 128], FP8, isOutput=False)
    wv = nc.declare_dram_parameter("wv", [128, NDIN * 128], FP8, isOutput=False)
    bqk = nc.declare_dram_parameter("bqk", [128, 2], F32, isOutput=False)
    bv = nc.declare_dram_parameter("bv", [1, 128], BF16, isOutput=False)
    wf = nc.declare_dram_parameter("wf", [128, NDIN * 1024], BF16, isOutput=False)
    wf8 = nc.declare_dram_parameter("wf8", [128, NDIN * 1024], FP8, isOutput=False)
    xres = nc.declare_dram_parameter("xres", [128, NDIN * 512], BF16, isOutput=False)
    out = nc.declare_dram_parameter("out", [512, D], F32, isOutput=True)

    with tile.TileContext(nc) as tc:
        with (
            tc.tile_pool(name="persist", bufs=1) as sbp,
            tc.tile_pool(name="dram", bufs=1, space="DRAM") as dram,
        ):
            # ---- persistent SBUF ----
            qt_sb = sbp.tile([128, VQ], FP8)  # [2h x 64dh, vq]
            kt_sb = sbp.tile([128, VQ], FP8)
            v8 = sbp.tile([128, 2 * NPAIR, 2, 160], FP8)  # (b,pair), sub, 2x80
            wq_sb = sbp.tile([128, NDIN, 128], FP8)
            wk_sb = sbp.tile([128, NDIN, 128], FP8)
            wv_sb = sbp.tile([128, NDIN, 128], FP8)
            bqk_sb = sbp.tile([128, 2], F32)
            bv_sb = sbp.tile([1, 128], BF16)
            bvb_sb = sbp.tile([128, 128], BF16)  # bv broadcast across partitions
            ones_bf = sbp.tile([1, 128], BF16)
            nc.vector.memset(ones_bf[:], 1.0)
            # rowsum ones columns: offset 64 within each 80-wide head slot
            nc.vector.memset(
                v8[:].rearrange("p a s (h n) -> p (a s h) n", n=80)[:, :, 64:65], 1.0
            )

            a2a_in = dram.tile([1024, 512], FP8)
            a2a_out = dram.tile([1024, 512], FP8)

            # ---- input DMAs ----
            # scalar queue: small weights needed immediately
            nc.scalar.dma_start(wk_sb[:], wk[:].rearrange("p (t n) -> p t n", t=NDIN))
            nc.scalar.dma_start(wv_sb[:], wv[:].rearrange("p (t n) -> p t n", t=NDIN))
            nc.scalar.dma_start(wq_sb[:], wq[:].rearrange("p (t n) -> p t n", t=NDIN))
            nc.scalar.dma_start(bqk_sb[:], bqk[:])
            nc.scalar.dma_start(bv_sb[:], bv[:])
            nc.gpsimd.partition_broadcast(bvb_sb[:], bv_sb[:])

            # sync queue: big x streams, k/v-first so attention starts early
            xst = tc.alloc_tile_pool(name="xst", bufs=2)
            xk_b, xv_b, xq_b = [], [], []
            for b in range(B):
                for nm, lst in (("xk", xk_b), ("xv", xv_b), ("xq", xq_b)):
                    lst.append(
                        xst.tile([128, NDIN, 2048], FP8, name=f"{nm}{b}", tag=nm)
                    )
            for b, eng in ((0, nc.sync), (1, nc.gpsimd)):
                for lst, srct in ((xk_b, xk), (xv_b, xv), (xq_b, xq)):
                    eng.dma_start(
                        lst[b][:],
                        srct[:, 2048 * b : 2048 * (b + 1)].rearrange(
                            "(t p) v -> p t v", p=128
                        ),
                    )

            # ---- projection helpers ----
            def qk_proj_emit(qkps, dst, w_sb, bcol, xt, b, win, copy_engine):
                """One 512-q window of the Q or K projection (fp8 DoubleRow)."""
                q0 = 2048 * b + 512 * win
                ps = qkps.tile([128, 512], F32, name=f"qkp{b}_{win}", tag="qkps")
                for dp in range(4):
                    nc.tensor.matmul(
                        ps[:],
                        lhsT=w_sb[:, 2 * dp : 2 * dp + 2, :],
                        rhs=xt[:, 2 * dp : 2 * dp + 2, 512 * win : 512 * win + 512],
                        start=(dp == 0),
                        stop=(dp == 3),
                        perf_mode=DR,
                    )
                if copy_engine == "act":
                    nc.scalar.activation(
                        dst[:, q0 : q0 + 512],
                        ps[:],
                        AF.Identity,
                        bias=bqk_sb[:, bcol : bcol + 1],
                    )
                else:
                    nc.vector.tensor_scalar_add(
                        dst[:, q0 : q0 + 512], ps[:], bqk_sb[:, bcol : bcol + 1]
                    )

            def v_unit(vps, b, kt):
                vp = vps.tile([128, 128], F32, name=f"vp{b}_{kt}", tag="vps")
                for dp in range(4):
                    nc.tensor.matmul(
                        vp[:],
                        lhsT=xv_b[b][:, 2 * dp : 2 * dp + 2, 128 * kt : 128 * (kt + 1)],
                        rhs=wv_sb[:, 2 * dp : 2 * dp + 2, :],
                        start=(dp == 0),
                        stop=(dp == 3),
                        perf_mode=DR,
                    )
                slot = NPAIR * b + kt // 2
                nc.vector.tensor_tensor(
                    v8[:, slot, kt % 2, :].rearrange("p (h n) -> p h n", h=2)[
                        :, :, 0:64
                    ],
                    vp[:].rearrange("p (h n) -> p h n", n=64),
                    bvb_sb[:].rearrange("p (h n) -> p h n", n=64),
                    ALU.add,
                )

            # ================= phase 1: batch-0 projections =================
            with (
                tc.tile_pool(name="qkps1", bufs=2, space="PSUM") as qkps1,
                tc.tile_pool(name="vps1", bufs=2, space="PSUM") as vps1,
            ):
                warm = vps1.tile([64, 64], F32, name="warm", tag="warm")
                for _ in range(100):
                    nc.tensor.matmul(
                        warm[:], lhsT=ones_bf[0:1, 0:64], rhs=ones_bf[0:1, 0:64],
                        start=True, stop=True,
                    )
                for win in range(4):
                    qk_proj_emit(qkps1, kt_sb, wk_sb, 1, xk_b[0], 0, win, "act")
                for kt in range(NKT):
                    v_unit(vps1, 0, kt)
                for win in range(4):
                    qk_proj_emit(qkps1, qt_sb, wq_sb, 0, xq_b[0], 0, win, "act")

            # ================= phase 2: attention =================
            with (
                tc.tile_pool(name="ptp", bufs=9) as ptp,
                tc.tile_pool(name="nrm", bufs=2) as nrm,
                tc.tile_pool(name="ztp", bufs=4) as ztp,
            ):
                stp_box = [None]
                def emit_qk(qc, kt, pt):
                    """Scores for (qc, kt) -> exp into pt[:, kt%2, :]."""
                    b = qc // 4
                    q0 = 512 * qc
                    kk = 2048 * b + 128 * kt
                    st = stp_box[0].tile(
                        [128, 1024], F32, name=f"st{qc}_{kt}", tag="st"
                    )
                    for h in range(2):
                        nc.tensor.matmul(
                            st[:, 512 * h : 512 * (h + 1)],
                            lhsT=kt_sb[64 * h : 64 * (h + 1), kk : kk + 128],
                            rhs=qt_sb[64 * h : 64 * (h + 1), q0 : q0 + 512],
                            start=True,
                            stop=True,
                        )
                    if kt in ACT_KT:
                        nc.scalar.activation(
                            pt[:, kt % 2, :], st[:], AF.Exp, scale=CEXP
                        )
                    else:
                        nc.vector.tensor_scalar(
                            pt[:, kt % 2, :].bitcast(I8),
                            st[:],
                            EXP_C1,
                            EXP_C2,
                            ALU.mult,
                            ALU.add,
                        )

                def emit_qk_pair(qc, pair):
                    pt = ptp.tile([128, 2, 1024], FP8, name=f"pt{qc}_{pair}", tag="pt")
                    emit_qk(qc, 2 * pair, pt)
                    emit_qk(qc, 2 * pair + 1, pt)
                    return pt

                def emit_pv(qc, pair, pt, ot0, ot1):
                    b = qc // 4
                    slot = NPAIR * b + pair
                    for h, ot in ((0, ot0), (1, ot1)):
                        nc.tensor.matmul(
                            ot[:],
                            lhsT=v8[:, slot, :, 80 * h : 80 * h + 65],
                            rhs=pt[:, :, 512 * h : 512 * (h + 1)],
                            start=(pair == 0),
                            stop=(pair == NPAIR - 1),
                            perf_mode=DR,
                        )

                def normalize(qc, ot0, ot1):
                    osb = nrm.tile([64, 1024], F32, name=f"osb{qc}", tag="osb")
                    rsf = nrm.tile([1, 1024], F32, name=f"rsf{qc}", tag="rsf")
                    for h, ot in ((0, ot0), (1, ot1)):
                        nc.scalar.copy(osb[:, 512 * h : 512 * (h + 1)], ot[0:64, :])
                        nc.scalar.copy(rsf[:, 512 * h : 512 * (h + 1)], ot[64:65, :])
                    rr = nrm.tile([1, 1024], F32, name=f"rr{qc}", tag="rr")
                    nc.vector.reciprocal_approx_fast(rr[:], rsf[:])
                    zbc = nrm.tile([64, 1024], F32, name=f"zbc{qc}", tag="zbc")
                    nc.gpsimd.partition_broadcast(zbc[:], rr[:])
                    for h in range(2):
                        zt = ztp.tile([64, 512], FP8, name=f"zt{qc}_{h}", tag="zt")
                        nc.gpsimd.tensor_tensor(
                            zt[:],
                            osb[:, 512 * h : 512 * (h + 1)],
                            zbc[:, 512 * h : 512 * (h + 1)],
                            ALU.mult,
                        )
                        nc.gpsimd.dma_start(
                            a2a_in[128 * qc + 64 * h : 128 * qc + 64 * h + 64, :],
                            zt[:],
                        )

                # batch-1 projection units, spread across the early chunks
                b1_units = []
                otp = tc.alloc_tile_pool(name="otp", bufs=1, space="PSUM")
                stp1 = tc.alloc_tile_pool(name="stp", bufs=2, space="PSUM")
                stp_box[0] = stp1
                with (
                    tc.tile_pool(name="qkps2", bufs=1, space="PSUM") as qkps2,
                    tc.tile_pool(name="vps2", bufs=1, space="PSUM") as vps2,
                ):
                    for win in range(4):
                        b1_units.append(
                            lambda w=win: qk_proj_emit(
                                qkps2, kt_sb, wk_sb, 1, xk_b[1], 1, w, "act"
                            )
                        )
                    for kt in range(NKT):
                        b1_units.append(lambda k=kt: v_unit(vps2, 1, k))
                    for win in range(4):
                        b1_units.append(
                            lambda w=win: qk_proj_emit(
                                qkps2, qt_sb, wq_sb, 0, xq_b[1], 1, w, "dve"
                            )
                        )
                    ui = 0

                    # prologue: qc0 scores interleaved with early b1 units
                    pts = []
                    for pair in range(NPAIR):
                        pts.append(emit_qk_pair(0, pair))
                        if ui < 8:
                            b1_units[ui]()
                            ui += 1

                    pending = None
                    for qc in range(2):
                        ot0 = otp.tile([65, 512], F32, name=f"ot0_{qc}", tag="ot0")
                        ot1 = otp.tile([65, 512], F32, name=f"ot1_{qc}", tag="ot1")
                        nxt = []
                        for pair in range(NPAIR):
                            nxt.append(emit_qk_pair(qc + 1, pair))
                            emit_pv(qc, pair, pts[pair], ot0, ot1)
                            if pair == 1 and pending is not None:
                                normalize(*pending)
                                pending = None
                            if ui < len(b1_units):
                                b1_units[ui]()
                                ui += 1
                        if pending is not None:
                            normalize(*pending)
                        pending = (qc, ot0, ot1)
                        pts = nxt

                stp1.release()
                with tc.tile_pool(name="stp2", bufs=3, space="PSUM") as stp2:
                    stp_box[0] = stp2
                    for qc in range(2, NQC):
                        ot0 = otp.tile([65, 512], F32, name=f"ot0_{qc}", tag="ot0")
                        ot1 = otp.tile([65, 512], F32, name=f"ot1_{qc}", tag="ot1")
                        nxt = []
                        for pair in range(NPAIR):
                            if qc + 1 < NQC:
                                nxt.append(emit_qk_pair(qc + 1, pair))
                            emit_pv(qc, pair, pts[pair], ot0, ot1)
                            if pair == 1 and pending is not None:
                                normalize(*pending)
                                pending = None
                        if pending is not None:
                            normalize(*pending)
                        pending = (qc, ot0, ot1)
                        pts = nxt
                    normalize(*pending)
                otp.release()

            xst.release()

            # ================= phase 3: fc-stage tensors + Yres + A2A + fc ==
            late = tc.alloc_tile_pool(name="late", bufs=1)
            wf_sb = late.tile([128, NDIN, 1024], BF16, name="wf_sb")
            wf8_sb = late.tile([128, NDIN, 1024], FP8, name="wf8_sb")
            xres_sb = late.tile([128, NDIN, 512], BF16, name="xres_sb")
            yres_sb = late.tile([128, 4, 1024], BF16, name="yres_sb")
            zf_sb = late.tile([128, NDIN, 512], FP8, name="zf_sb")
            nc.sync.dma_start(xres_sb[:], xres[:].rearrange("p (t n) -> p t n", t=NDIN))
            nc.sync.dma_start(wf_sb[:], wf[:].rearrange("p (t n) -> p t n", t=NDIN))
            nc.sync.dma_start(wf8_sb[:], wf8[:].rearrange("p (t n) -> p t n", t=NDIN))

            with (
                tc.tile_pool(name="fcps", bufs=4, space="PSUM") as fcps,
                tc.tile_pool(name="ysb", bufs=2) as ysb,
            ):
                for qt in range(4):
                    for nb in range(2):
                        yp = fcps.tile([128, 512], F32, name=f"yr{qt}_{nb}", tag="yr")
                        for j in range(NDIN):
                            nc.tensor.matmul(
                                yp[:],
                                lhsT=xres_sb[:, j, 128 * qt : 128 * (qt + 1)],
                                rhs=wf_sb[:, j, 512 * nb : 512 * (nb + 1)],
                                start=(j == 0),
                                stop=(j == NDIN - 1),
                            )
                        nc.vector.tensor_copy(
                            yres_sb[:, qt, 512 * nb : 512 * (nb + 1)], yp[:]
                        )

                nc.gpsimd.collective_compute(
                    "AllToAll",
                    ALU.bypass,
                    replica_groups=[list(range(N_CORES))],
                    ins=[a2a_in.opt()],
                    outs=[a2a_out.opt()],
                )
                nc.sync.dma_start(
                    zf_sb[:], a2a_out[:].rearrange("(t p) v -> p t v", p=128)
                )
                # attention half (fp8 DoubleRow) + residual add + store
                for qt in range(4):
                    y = ysb.tile([128, 1024], F32, name=f"y{qt}", tag="y")
                    for nb in range(2):
                        yp = fcps.tile([128, 512], F32, name=f"ya{qt}_{nb}", tag="ya")
                        for dp in range(4):
                            nc.tensor.matmul(
                                yp[:],
                                lhsT=zf_sb[:, 2 * dp : 2 * dp + 2, 128 * qt : 128 * (qt + 1)],
                                rhs=wf8_sb[:, 2 * dp : 2 * dp + 2, 512 * nb : 512 * (nb + 1)],
                                start=(dp == 0),
                                stop=(dp == 3),
                                perf_mode=DR,
                            )
                        nc.vector.scalar_tensor_tensor(
                            y[:, 512 * nb : 512 * (nb + 1)],
                            yp[:],
                            1.0 / (WSCALE * W2),
                            yres_sb[:, qt, 512 * nb : 512 * (nb + 1)],
                            ALU.mult,
                            ALU.add,
                        )
                    nc.sync.dma_start(out[128 * qt : 128 * (qt + 1), :], y[:])

            late.release()

    nc.compile()
    return nc


def _to_sbuf_layout(w, inner):
    """[D, N] -> [128, NDIN*inner] with w_pre[p, t*inner+n] = w[128t+p, n]."""
    return np.ascontiguousarray(
        w.reshape(NDIN, 128, inner).transpose(1, 0, 2).reshape(128, NDIN * inner)
    )


def _numpy_reference(q_, k_, v_, mask, Wq, bq, Wk, bk, Wv, bv, Wf, bf):
    q_ = np.asarray(q_, np.float32)
    k_ = np.asarray(k_, np.float32)
    v_ = np.asarray(v_, np.float32)
    b = q_.shape[0]

    def split(x):
        return x.reshape(b, -1, H, DH).transpose(0, 2, 1, 3)

    q = split(q_ @ Wq + bq)
    k = split(k_ @ Wk + bk)
    v = split(v_ @ Wv + bv)
    attn = np.einsum("bhqd,bhkd->bhqk", q, k) / np.sqrt(np.float32(DH))
    attn = np.where(np.asarray(mask)[:, None, :, None], attn, np.float32(-1e12))
    attn = attn - attn.max(axis=-1, keepdims=True)
    e = np.exp(attn)
    p = e / e.sum(axis=-1, keepdims=True)
    o = np.einsum("bhqk,bhkd->bhqd", p, v)
    o = o.transpose(0, 2, 1, 3).reshape(b, -1, D)
    return (o + q_) @ Wf + bf


def kernel(q_, k_, v_, mask, Wq, bq, Wk, bk, Wv, bv, Wf, bf):
    mask = np.asarray(mask)
    if not mask.all():
        return _numpy_reference(q_, k_, v_, mask, Wq, bq, Wk, bk, Wv, bv, Wf, bf)

    q_ = np.asarray(q_, np.float32)
    k_ = np.asarray(k_, np.float32)
    v_ = np.asarray(v_, np.float32)
    Wq = np.asarray(Wq, np.float32)
    Wk = np.asarray(Wk, np.float32)
    Wv = np.asarray(Wv, np.float32)
    Wf = np.asarray(Wf, np.float32)
    bq = np.asarray(bq, np.float32)
    bk = np.asarray(bk, np.float32)
    bv = np.asarray(bv, np.float32)

    xq8 = np.ascontiguousarray(
        np.concatenate([q_[b].T for b in range(B)], axis=1)
    ).astype(FP8NP)
    xk8 = np.ascontiguousarray(
        np.concatenate([k_[b].T for b in range(B)], axis=1)
    ).astype(FP8NP)
    xv8 = np.ascontiguousarray(
        np.concatenate([v_[b].T for b in range(B)], axis=1)
    ).astype(FP8NP)
    wf_b = _to_sbuf_layout(Wf, 1024).astype(BF16NP)
    wf8_b = _to_sbuf_layout(Wf * W2, 1024).astype(FP8NP)

    in_maps = []
    for c in range(N_CORES):
        d0 = 128 * c
        in_maps.append(
            {
                "xq": xq8,
                "xk": xk8,
                "xv": xv8,
                "wq": _to_sbuf_layout(Wq[:, d0 : d0 + 128] * WSCALE, 128).astype(
                    FP8NP
                ),
                "wk": _to_sbuf_layout(Wk[:, d0 : d0 + 128] * WSCALE, 128).astype(
                    FP8NP
                ),
                "wv": _to_sbuf_layout(Wv[:, d0 : d0 + 128] * WSCALE, 128).astype(
                    FP8NP
                ),
                "bqk": np.ascontiguousarray(
                    np.stack(
                        [bq[d0 : d0 + 128] * WSCALE, bk[d0 : d0 + 128] * WSCALE],
                        axis=1,
                    )
                ),
                "bv": np.ascontiguousarray(
                    bv[None, d0 : d0 + 128] * WSCALE
                ).astype(BF16NP),
                "wf": wf_b,
                "wf8": wf8_b,
                "xres": _to_sbuf_layout(
                    np.ascontiguousarray(
                        q_[c // 4].T[:, 512 * (c % 4) : 512 * (c % 4 + 1)]
                    ),
                    512,
                ).astype(BF16NP),
            }
        )

    if "nc" not in _CACHE:
        _CACHE["nc"] = _build()
    res = run_bass_kernel_spmd(_CACHE["nc"], in_maps, core_ids=list(range(N_CORES)))

    out = np.empty((B, S, D), np.float32)
    for c in range(N_CORES):
        y = res.results[c]["out"]
        out[c // 4, 512 * (c % 4) : 512 * (c % 4 + 1), :] = y
    out += np.asarray(bf, np.float32)[None, None, :]
    return out


if __name__ == "__main__":
    rng = np.random.default_rng(0)
    args = dict(
        q_=rng.standard_normal((B, S, D), dtype=np.float32),
        k_=rng.standard_normal((B, S, D), dtype=np.float32),
        v_=rng.standard_normal((B, S, D), dtype=np.float32),
        mask=np.ones((B, S), bool),
        Wq=rng.standard_normal((D, D), dtype=np.float32) * 0.02,
        bq=np.zeros(D, np.float32),
        Wk=rng.standard_normal((D, D), dtype=np.float32) * 0.02,
        bk=np.zeros(D, np.float32),
        Wv=rng.standard_normal((D, D), dtype=np.float32) * 0.02,
        bv=np.zeros(D, np.float32),
        Wf=rng.standard_normal((D, D), dtype=np.float32) * 0.02,
        bf=np.zeros(D, np.float32),
    )
    got = kernel(**args)
    want = _numpy_reference(**args)
    rel = np.abs(got - want).max() / np.abs(want).max()
    print("rel_err:", rel)
